# revision 5
# baseline (speedup 1.0000x reference)
"""Trainium2 Bass kernel for the token-scan problem (bf16 rewrite).

Math: the scan over T=128 tokens unrolls into dense matmuls:

  V     = token_emb[tokens]                    [T, d]
  R     = relu(Dx @ V^T)                       [n, T]  (rcols: [n,T], rt: [T,n])
  x_f   = R @ 1_T                              [n]     (accum_out of the relus)
  h     = R^T x_f                              [T]
  a*    = (U diag(w) V)^T h      (= vwc^T h)   [d]     -> AllReduce (with mean)
  yc    = Dy @ ln(a*) = (Dy a*)/s - (m/s)(Dy 1)
  y     = relu(yc) * x_f                       [n]
  vs    = ln(E @ y)                            [d]     -> AllReduce (with mean)
  rho   = (U diag(wp) V)^T @ R^T               [d, n]

Sharding: n split across 8 cores.  Two AllReduces of a 257-float vector
(payload + pre-reduced mean).  rho is computed between the two AllReduces
where the engines otherwise idle; each AllReduce window is kept free of
retiring instructions.
"""

import numpy as np

N, D, V_VOCAB, T = 16384, 256, 32000, 128
DECAY = 0.97
N_CORES = 8
NS = N // N_CORES           # 2048 rows per core
NT = NS // 128              # 16 tiles of 128
NQ = NS // 512              # 4 chunks of 512

N_DUMMY = 6                 # PE warm-up matmuls at t=0

_cache = {}
STAGE = 99


def _build():
    stage = STAGE
    import concourse.bacc as bacc
    import concourse.mybir as mybir
    import concourse.tile as tile

    f32 = mybir.dt.float32
    bf16 = mybir.dt.bfloat16
    AF = mybir.ActivationFunctionType
    ALU = mybir.AluOpType

    nc = bacc.Bacc("TRN2", target_bir_lowering=False, debug=False,
                   num_devices=N_CORES)

    # ---- dram inputs (per-core, host-prepped layouts) ----
    # dxq: 4 quarters, each [128, 1024] = both d-halves of 512 n-rows
    i_dxq = [nc.dram_tensor(f"dxq{q}", [128, 1024], bf16, kind="ExternalInput")
             for q in range(4)]
    i_dyts = nc.dram_tensor("dyts", [128, 2 * NS], bf16, kind="ExternalInput")
    i_ets = nc.dram_tensor("ets", [128, NT * 256], bf16, kind="ExternalInput")
    # cbf = [vts(256) | vwp(256) | vwc_ext(257) | dy1col(16)]
    i_cbf = nc.dram_tensor("cbf", [128, 785], bf16, kind="ExternalInput")
    # cf32: col0 = ones column; row0 cols 2..130 = ones row
    i_cf32 = nc.dram_tensor("cf32", [128, 130], f32, kind="ExternalInput")

    o_x = nc.dram_tensor("out_x", [NS], f32, kind="ExternalOutput")
    o_y = nc.dram_tensor("out_y", [NS], bf16, kind="ExternalOutput")
    o_vs = nc.dram_tensor("out_vs", [256], f32, kind="ExternalOutput")
    o_rho = nc.dram_tensor("out_rho", [256, NS], bf16, kind="ExternalOutput")

    with tile.TileContext(nc) as tc:
        with (
            tc.tile_pool(name="persist", bufs=1) as pp,
            tc.tile_pool(name="work", bufs=2) as wp,
            tc.tile_pool(name="psrc", bufs=2, space="PSUM") as psrc,
            tc.tile_pool(name="psrt", bufs=2, space="PSUM") as psrt,
            tc.tile_pool(name="psha", bufs=1, space="PSUM") as psha,
            tc.tile_pool(name="psb", bufs=1, space="PSUM") as psb,
            tc.tile_pool(name="psx", bufs=1, space="PSUM") as psx,
            tc.tile_pool(name="psd", bufs=1, space="PSUM") as psd,
            tc.tile_pool(name="dram", bufs=1, space="DRAM") as dram,
        ):
            # ---------------- loads ----------------
            dxts = pp.tile([128, 4096], bf16)       # [q0d0 q0d1 q1d0 q1d1 ..]
            cbf = pp.tile([128, 785], bf16)
            cf32 = pp.tile([128, 130], f32)
            dyts = pp.tile([128, 2 * NS], bf16)
            ets = pp.tile([128, NT * 256], bf16)
            dm = pp.tile([128, 512], bf16)          # dummy-warmup operand

            nc.sync.dma_start(dxts[:, 0:1024], i_dxq[0][:])
            nc.gpsimd.dma_start(dxts[:, 1024:2048], i_dxq[1][:])
            nc.scalar.dma_start(cbf[:], i_cbf[:])
            nc.scalar.dma_start(dxts[:, 2048:3072], i_dxq[2][:])
            nc.sync.dma_start(dxts[:, 3072:4096], i_dxq[3][:])
            nc.scalar.dma_start(cf32[:], i_cf32[:])
            nc.vector.memset(dm[:], 0.0)
            # bulk loads for the post-AR phases; queues are free meanwhile
            nc.sync.dma_start(dyts[:], i_dyts[:])
            nc.gpsimd.dma_start(ets[:], i_ets[:])

            vts = cbf[:, 0:256]
            vwp = cbf[:, 256:512]
            vwc = cbf[:, 512:769]      # [128, 257] incl. mean column
            dy1col = cbf[:, 769:785]
            ones_col = cf32[:, 0:1]
            ones_row = cf32[0:1, 2:130]

            # ---------------- PE warm-up ----------------
            for k in range(N_DUMMY):
                dm_ps = psd.tile([128, 512], f32, tag="dm")
                nc.tensor.matmul(dm_ps[:], lhsT=dm[:, 0:128], rhs=dm[:],
                                 start=True, stop=True)

            # ---------------- phase A: rcols -> h -> a* ----------------
            # rcols_i = relu(Dx_i V^T) [128n, 128T], fp32 in SBUF;
            # relu accum gives xfcol columns for free.
            rcols = pp.tile([128, NT * 128], f32)
            xfcol = pp.tile([128, NT], f32)
            h_done = []
            qorder = [0, 1, 3, 2]
            for gi, q in enumerate(qorder):
                rc_ps = psrc.tile([128, 512], f32, tag="rc")
                for j in range(4):
                    i = q * 4 + j
                    for c in range(2):
                        nc.tensor.matmul(
                            rc_ps[:, j * 128:(j + 1) * 128],
                            lhsT=dxts[:, q * 1024 + c * 512 + j * 128:
                                      q * 1024 + c * 512 + (j + 1) * 128],
                            rhs=vts[:, c * 128:(c + 1) * 128],
                            start=(c == 0), stop=(c == 1))
                # relu+accum, alternating DVE / Act per tile
                for j in range(4):
                    i = q * 4 + j
                    src = rc_ps[:, j * 128:(j + 1) * 128]
                    dst = rcols[:, i * 128:(i + 1) * 128]
                    if (gi * 4 + j) % 2 == 0:
                        nc.vector.tensor_scalar(
                            dst, src, 0.0, None, ALU.max, op1=ALU.add,
                            accum_out=xfcol[:, i:i + 1])
                    else:
                        nc.scalar.activation(dst, src, AF.Relu,
                                             accum_out=xfcol[:, i:i + 1])

            if stage >= 2:
                # h = sum_i rcols_i^T xfcol_i  (one PSUM bank accum chain)
                h_ps = psha.tile([128, 1], f32, tag="ha")
                for gi, q in enumerate(qorder):
                    for j in range(4):
                        i = q * 4 + j
                        nc.tensor.matmul(
                            h_ps[:],
                            lhsT=rcols[:, i * 128:(i + 1) * 128],
                            rhs=xfcol[:, i:i + 1],
                            start=(gi == 0 and j == 0),
                            stop=(gi == 3 and j == 3))
                h_sb = pp.tile([128, 1], bf16)
                nc.vector.tensor_copy(h_sb[:], h_ps[:])
                # a_ext = vwc_ext^T h : [1, 257] (col 256 = mean of a)
                a_ps = psha.tile([1, 257], f32, tag="ha")
                nc.tensor.matmul(a_ps[:], lhsT=h_sb[:], rhs=vwc[:],
                                 start=True, stop=True)
                a_sb = pp.tile([1, 257], f32)
                nc.scalar.activation(a_sb[:], a_ps[:], AF.Copy)
                # x_f out (fp32)
                nc.scalar.dma_start(o_x[:].rearrange("(i p) -> p i", p=128),
                                    xfcol[:])

            if stage >= 3:
                # ---- AllReduce #1 (a_ext) ----
                a_in = dram.tile([1, 257], f32)
                a_out = dram.tile([1, 257], f32)
                nc.sync.dma_start(a_in[:], a_sb[:])
                nc.gpsimd.collective_compute(
                    "AllReduce", ALU.add,
                    replica_groups=[list(range(N_CORES))],
                    ins=[a_in.opt()], outs=[a_out.opt()],
                )

            # rt = relu(V Dx^T) [T, n] bf16 (only needed for rho in phase B;
            # emitted here so it runs pre-AR on PE/Act/DVE slack)
            rt = pp.tile([128, NS], bf16)
            for q in range(NQ):
                rt_ps = psrt.tile([128, 512], f32, tag="rt")
                for c in range(2):
                    nc.tensor.matmul(
                        rt_ps[:],
                        lhsT=vts[:, c * 128:(c + 1) * 128],
                        rhs=dxts[:, q * 1024 + c * 512: q * 1024 + (c + 1) * 512],
                        start=(c == 0), stop=(c == 1))
                if q % 2 == 0:
                    nc.vector.tensor_scalar(rt[:, q * 512:(q + 1) * 512],
                                            rt_ps[:], 0.0, None, ALU.max)
                else:
                    nc.scalar.activation(rt[:, q * 512:(q + 1) * 512],
                                         rt_ps[:], AF.Relu)

            if stage >= 4:
                # ---- phase B: everything behind AllReduce #1 ----
                # acol [128,2]: a_d at [d%128, d//128]; mrow [1,1] = mean
                acol = pp.tile([128, 2], f32)
                mrow = pp.tile([1, 1], f32)
                nc.sync.dma_start(
                    acol[:], a_out[0:1, 0:256].rearrange("a (c p) -> (a p) c", p=128))
                nc.scalar.dma_start(mrow[:], a_out[0:1, 256:257])
                acolb = pp.tile([128, 2], bf16)
                nc.vector.tensor_copy(acolb[:], acol[:])

                # yca = Dy @ a  [128, 16] (in-order PE queue blocks on acolb)
                yca_ps = psb.tile([128, NT], f32, tag="b")
                for i in range(NT):
                    for c in range(2):
                        nc.tensor.matmul(
                            yca_ps[:, i:i + 1],
                            lhsT=dyts[:, c * NS + i * 128: c * NS + (i + 1) * 128],
                            rhs=acolb[:, c:c + 1],
                            start=(c == 0), stop=(c == 1))

                # sigma chain: std = sqrt((sum a^2 - 256 m^2)/255)
                sq = pp.tile([128, 2], f32)
                pssq = pp.tile([128, 1], f32)
                nc.vector.tensor_tensor_reduce(
                    out=sq[:], in0=acol[:], in1=acol[:], scale=1.0, scalar=0.0,
                    op0=ALU.mult, op1=ALU.add, accum_out=pssq[:])
                ssq_ps = psx.tile([1, 1], f32, tag="x")
                nc.tensor.matmul(ssq_ps[:], lhsT=pssq[:], rhs=ones_col[:],
                                 start=True, stop=True)
                msq = pp.tile([1, 1], f32)
                msq_o = pp.tile([1, 1], f32)
                nc.vector.tensor_tensor_reduce(
                    out=msq_o[:], in0=mrow[:], in1=mrow[:],
                    scale=-256.0 / 255.0, scalar=0.0,
                    op0=ALU.mult, op1=ALU.add, accum_out=msq[:])
                std = pp.tile([1, 1], f32)
                nc.scalar.activation(std[:], ssq_ps[:], AF.Sqrt,
                                     bias=msq[:], scale=1.0 / 255.0)
                scal2 = pp.tile([1, 2], f32)
                nc.vector.reciprocal(scal2[:, 0:1], std[:])
                nc.vector.tensor_mul(scal2[:, 1:2], mrow[:], scal2[:, 0:1])
                bc_ps = psx.tile([128, 2], f32, tag="x")
                nc.tensor.matmul(bc_ps[:], lhsT=ones_row[:], rhs=scal2[:],
                                 start=True, stop=True)
                cols = pp.tile([128, 2], f32)
                nc.vector.tensor_copy(cols[:], bc_ps[:])

                # y = relu(yca/s - (m/s) dy1) * x_f
                t1 = pp.tile([128, NT], f32)
                nc.vector.tensor_scalar(t1[:], yca_ps[:], cols[:, 0:1], None,
                                        ALU.mult)
                u2 = pp.tile([128, NT], f32)
                nc.gpsimd.tensor_scalar(u2[:], dy1col[:], cols[:, 1:2], None,
                                        ALU.mult)
                ysub = pp.tile([128, NT], f32)
                nc.vector.tensor_sub(ysub[:], t1[:], u2[:])
                ycr = pp.tile([128, NT], f32)
                nc.vector.tensor_scalar_max(ycr[:], ysub[:], 0.0)
                y = pp.tile([128, NT], bf16)
                nc.vector.tensor_mul(y[:], ycr[:], xfcol[:])
                nc.scalar.dma_start(o_y[:].rearrange("(i p) -> p i", p=128),
                                    y[:])

            if stage >= 5:
                # rho = vwp^T @ rt  [256, n]; runs on PE after yca (in-order),
                # i.e. inside the phase-B window, never inside an AR window.
                for dc in range(2):
                    rho_sb = wp.tile([128, NS], bf16, tag="rho_sb")
                    for q in range(NQ):
                        rho_ps = psrc.tile([128, 512], f32, tag="rc")
                        nc.tensor.matmul(rho_ps[:],
                                         lhsT=vwp[:, dc * 128:(dc + 1) * 128],
                                         rhs=rt[:, q * 512:(q + 1) * 512],
                                         start=True, stop=True)
                        dst = rho_sb[:, q * 512:(q + 1) * 512]
                        if dc == 0:
                            nc.scalar.activation(dst, rho_ps[:], AF.Copy)
                        else:
                            nc.vector.tensor_copy(dst, rho_ps[:])
                    nc.sync.dma_start(o_rho[dc * 128:(dc + 1) * 128, :],
                                      rho_sb[:])

            if stage >= 6:
                # vs_partial = y^T E^T : [1, 256] + pre-reduced mean col
                vs_ps = psb.tile([1, 256], f32, tag="b")
                for i in range(NT):
                    nc.tensor.matmul(vs_ps[:],
                                     lhsT=y[:, i:i + 1],
                                     rhs=ets[:, i * 256:(i + 1) * 256],
                                     start=(i == 0), stop=(i == NT - 1))
                vs_sb = pp.tile([1, 257], f32)
                vs_scr = pp.tile([1, 256], f32)
                nc.vector.tensor_copy(vs_sb[:, 0:256], vs_ps[:])
                nc.scalar.activation(vs_scr[:], vs_ps[:], AF.Copy,
                                     scale=1.0 / 256.0,
                                     accum_out=vs_sb[:, 256:257])

                # ---- AllReduce #2 (vs_ext) ----
                vs_in = dram.tile([1, 257], f32)
                vs_out = dram.tile([1, 257], f32)
                nc.sync.dma_start(vs_in[:], vs_sb[:])
                nc.gpsimd.collective_compute(
                    "AllReduce", ALU.add,
                    replica_groups=[list(range(N_CORES))],
                    ins=[vs_in.opt()], outs=[vs_out.opt()],
                )

            if stage >= 7:
                # ---- phase C: final layernorm of vs ----
                vrow = pp.tile([1, 257], f32)
                nc.sync.dma_start(vrow[:], vs_out[:])
                cen = pp.tile([1, 256], f32)
                nc.vector.tensor_scalar(cen[:], vrow[:, 0:256],
                                        vrow[:, 256:257], None, ALU.subtract)
                vsq = pp.tile([1, 256], f32)
                vssq = pp.tile([1, 1], f32)
                nc.vector.tensor_tensor_reduce(
                    out=vsq[:], in0=cen[:], in1=cen[:], scale=1.0 / 255.0,
                    scalar=0.0, op0=ALU.mult, op1=ALU.add, accum_out=vssq[:])
                vstd = pp.tile([1, 1], f32)
                nc.scalar.activation(vstd[:], vssq[:], AF.Sqrt)
                vinv = pp.tile([1, 1], f32)
                nc.vector.reciprocal(vinv[:], vstd[:])
                vsln = pp.tile([1, 256], f32)
                nc.vector.tensor_scalar(vsln[:], cen[:], vinv[:], None,
                                        ALU.mult)
                nc.sync.dma_start(o_vs[:].rearrange("(a b) -> a b", a=1),
                                  vsln[:])

    nc.finalize()
    return nc


def _host_prep(E, Dx, Dy, token_emb, tokens):
    import ml_dtypes
    bf = ml_dtypes.bfloat16

    E = np.asarray(E, dtype=np.float32)
    Dx = np.asarray(Dx, dtype=np.float32)
    Dy = np.asarray(Dy, dtype=np.float32)
    token_emb = np.asarray(token_emb, dtype=np.float32)
    tokens = np.asarray(tokens).astype(np.int64)

    v = np.ascontiguousarray(token_emb[tokens])          # [T, d]
    j = np.arange(T)
    w = (DECAY ** ((T - 1) - j)).astype(np.float32)
    w[T - 1] = 0.0
    wp = (DECAY ** (T - j)).astype(np.float32)
    u = np.triu(np.ones((T, T), dtype=np.float32))
    vwc = u @ (v * w[:, None])                           # [T, d]
    vwc_ext = np.concatenate([vwc, vwc.sum(1, keepdims=True) / 256.0], axis=1)
    vwp = u @ (v * wp[:, None])                          # [T, d]
    vts = np.concatenate([v[:, :128].T, v[:, 128:].T], axis=1)   # [128, 256]

    # vwp/vwc are [T, d] with T=128 partitions already
    cbf = np.ascontiguousarray(np.concatenate([vts, vwp, vwc_ext], axis=1))

    cf32 = np.zeros((128, 130), dtype=np.float32)
    cf32[:, 0] = 1.0
    cf32[0, 2:130] = 1.0

    in_maps = []
    for k in range(N_CORES):
        sl = slice(k * NS, (k + 1) * NS)
        dx_s = Dx[sl]                                    # [NS, 256]
        dy_s = Dy[sl]
        e_s = E[:, sl]                                   # [256, NS]
        dxq = []
        for q in range(4):
            rs = slice(q * 512, (q + 1) * 512)
            dxq.append(np.ascontiguousarray(np.concatenate(
                [dx_s[rs, :128].T, dx_s[rs, 128:].T], axis=1)).astype(bf))
        dyts = np.concatenate([dy_s[:, :128].T, dy_s[:, 128:].T], axis=1)
        ets = np.concatenate(
            [e_s[:, i * 128:(i + 1) * 128].T for i in range(NT)], axis=1)
        dy1 = dy_s.sum(axis=1)                           # [NS]
        dy1col = dy1.reshape(NT, 128).T                  # [128, 16]
        cbf_k = np.concatenate([cbf, dy1col], axis=1).astype(bf)
        in_maps.append({
            "dxq0": dxq[0], "dxq1": dxq[1], "dxq2": dxq[2], "dxq3": dxq[3],
            "dyts": np.ascontiguousarray(dyts).astype(bf),
            "ets": np.ascontiguousarray(ets).astype(bf),
            "cbf": np.ascontiguousarray(cbf_k),
            "cf32": cf32,
        })
    return in_maps


def kernel(E, Dx, Dy, token_emb, tokens, _trace=False):
    from concourse.bass_utils import run_bass_kernel_spmd

    key = ("nc", STAGE)
    if key not in _cache:
        _cache[key] = _build()
    nc = _cache[key]

    in_maps = _host_prep(E, Dx, Dy, token_emb, tokens)
    res = run_bass_kernel_spmd(nc, in_maps, core_ids=list(range(N_CORES)),
                               trace=_trace)
    _cache["last_result"] = res

    r = res.results
    x_full = np.concatenate([r[k]["out_x"] for k in range(N_CORES)])
    y_full = np.concatenate(
        [r[k]["out_y"].astype(np.float32) for k in range(N_CORES)])
    vs = r[0]["out_vs"]
    rho = np.concatenate(
        [r[k]["out_rho"].astype(np.float32) for k in range(N_CORES)], axis=1)
    return np.concatenate([x_full, y_full, vs, rho.ravel()]).astype(np.float32)


# revision 9
# speedup vs baseline: 1.0163x; 1.0163x over previous
"""Trainium2 Bass kernel for the token-scan problem (bf16 rewrite).

Math: the scan over T=128 tokens unrolls into dense matmuls:

  V     = token_emb[tokens]                    [T, d]
  R     = relu(Dx @ V^T)                       [n, T]  (rcols: [n,T], rt: [T,n])
  x_f   = R @ 1_T                              [n]     (accum_out of the relus)
  h     = R^T x_f                              [T]
  a*    = (U diag(w) V)^T h      (= vwc^T h)   [d]     -> AllReduce (with mean)
  yc    = Dy @ ln(a*) = (Dy a*)/s - (m/s)(Dy 1)
  y     = relu(yc) * x_f                       [n]
  vs    = ln(E @ y)                            [d]     -> AllReduce (with mean)
  rho   = (U diag(wp) V)^T @ R^T               [d, n]

Sharding: n split across 8 cores.  Two AllReduces of a 257-float vector
(payload + pre-reduced mean).  rho is computed between the two AllReduces
where the engines otherwise idle; each AllReduce window is kept free of
retiring instructions.
"""

import numpy as np

N, D, V_VOCAB, T = 16384, 256, 32000, 128
DECAY = 0.97
N_CORES = 8
NS = N // N_CORES           # 2048 rows per core
NT = NS // 128              # 16 tiles of 128
NQ = NS // 512              # 4 chunks of 512

N_DUMMY = 6                 # PE warm-up matmuls at t=0

_cache = {}
STAGE = 99


def _build():
    stage = STAGE
    import concourse.bacc as bacc
    import concourse.mybir as mybir
    import concourse.tile as tile

    f32 = mybir.dt.float32
    bf16 = mybir.dt.bfloat16
    AF = mybir.ActivationFunctionType
    ALU = mybir.AluOpType

    nc = bacc.Bacc("TRN2", target_bir_lowering=False, debug=False,
                   num_devices=N_CORES)

    # ---- dram inputs (per-core, host-prepped layouts) ----
    # dxq: 4 quarters, each [128, 1024] = both d-halves of 512 n-rows
    i_dxq = [nc.dram_tensor(f"dxq{q}", [128, 1024], bf16, kind="ExternalInput")
             for q in range(4)]
    i_dyts = nc.dram_tensor("dyts", [128, 2 * NS], bf16, kind="ExternalInput")
    i_ets = nc.dram_tensor("ets", [128, NT * 256], bf16, kind="ExternalInput")
    # cbf = [vts(256) | vwp(256) | vwc_ext(257) | dy1col(16)]
    i_cbf = nc.dram_tensor("cbf", [128, 785], bf16, kind="ExternalInput")
    # cf32: col0 = ones column; row0 cols 2..130 = ones row
    i_cf32 = nc.dram_tensor("cf32", [128, 130], f32, kind="ExternalInput")

    o_x = nc.dram_tensor("out_x", [NS], f32, kind="ExternalOutput")
    o_y = nc.dram_tensor("out_y", [NS], bf16, kind="ExternalOutput")
    o_vs = nc.dram_tensor("out_vs", [256], f32, kind="ExternalOutput")
    o_rho = nc.dram_tensor("out_rho", [256, NS], bf16, kind="ExternalOutput")

    with tile.TileContext(nc) as tc:
        with (
            tc.tile_pool(name="persist", bufs=1) as pp,
            tc.tile_pool(name="work", bufs=2) as wp,
            tc.tile_pool(name="psrc", bufs=3, space="PSUM") as psrc,
            tc.tile_pool(name="psrt", bufs=2, space="PSUM") as psrt,
            tc.tile_pool(name="psha", bufs=1, space="PSUM") as psha,
            tc.tile_pool(name="psb", bufs=1, space="PSUM") as psb,
            tc.tile_pool(name="psx", bufs=1, space="PSUM") as psx,
            tc.tile_pool(name="dram", bufs=1, space="DRAM") as dram,
        ):
            # ---------------- loads ----------------
            dxts = pp.tile([128, 4096], bf16)       # [q0d0 q0d1 q1d0 q1d1 ..]
            cbf = pp.tile([128, 785], bf16)
            cf32 = pp.tile([128, 130], f32)
            dyts = pp.tile([128, 2 * NS], bf16)
            ets = pp.tile([128, NT * 256], bf16)
            dm = pp.tile([128, 512], bf16)          # dummy-warmup operand

            # SP: cbf first (gates all matmuls), then dx quarters.
            # Pool: dx quarters + ets.  Act: kept clear for the early relus
            # (the framework's act-table load head-blocks it anyway).
            nc.sync.dma_start(cbf[:], i_cbf[:])
            nc.gpsimd.dma_start(dxts[:, 1024:2048], i_dxq[1][:])
            nc.sync.dma_start(dxts[:, 0:1024], i_dxq[0][:])
            nc.gpsimd.dma_start(dxts[:, 2048:3072], i_dxq[2][:])
            nc.sync.dma_start(dxts[:, 3072:4096], i_dxq[3][:])
            nc.vector.memset(dm[:], 0.0)
            # bulk loads for the post-AR phases; queues are free meanwhile
            nc.sync.dma_start(dyts[:], i_dyts[:])
            nc.gpsimd.dma_start(ets[:], i_ets[:])

            vts = cbf[:, 0:256]
            vwp = cbf[:, 256:512]
            vwc = cbf[:, 512:769]      # [128, 257] incl. mean column
            dy1col = cbf[:, 769:785]
            ones_col = cf32[:, 0:1]
            ones_row = cf32[0:1, 2:130]

            # ---------------- PE warm-up ----------------
            for k in range(N_DUMMY):
                dm_ps = psrt.tile([128, 512], f32, tag="rt")
                nc.tensor.matmul(dm_ps[:], lhsT=dm[:, 0:128], rhs=dm[:],
                                 start=True, stop=True)

            # ---------------- phase A: rcols -> h -> a* ----------------
            # rcols_i = relu(Dx_i V^T) [128n, 128T], fp32 in SBUF;
            # relu accum gives xfcol columns for free.  Per-engine xf
            # accumulator tiles avoid cross-engine WAW serialization.
            N_DVE_RELU = 9      # tiles 0..8 on DVE, 9..15 on Act
            rcols = pp.tile([128, NT * 128], f32)
            xfA = pp.tile([128, NT], f32)
            xfB = pp.tile([128, NT], f32)
            xf_of = lambda i: (xfA if i < N_DVE_RELU else xfB)[:, i:i + 1]
            qorder = [1, 0, 2, 3]
            for gi, q in enumerate(qorder):
                rc_ps = psrc.tile([128, 512], f32, tag="rc")
                for j in range(4):
                    i = q * 4 + j
                    for c in range(2):
                        nc.tensor.matmul(
                            rc_ps[:, j * 128:(j + 1) * 128],
                            lhsT=dxts[:, q * 1024 + c * 512 + j * 128:
                                      q * 1024 + c * 512 + (j + 1) * 128],
                            rhs=vts[:, c * 128:(c + 1) * 128],
                            start=(c == 0), stop=(c == 1))
                for j in range(4):
                    i = q * 4 + j
                    src = rc_ps[:, j * 128:(j + 1) * 128]
                    dst = rcols[:, i * 128:(i + 1) * 128]
                    if i < N_DVE_RELU:
                        nc.vector.tensor_scalar(
                            dst, src, 0.0, None, ALU.max, op1=ALU.add,
                            accum_out=xf_of(i))
                    else:
                        nc.scalar.activation(dst, src, AF.Relu,
                                             accum_out=xf_of(i))

            if stage >= 2:
                # h = sum_i rcols_i^T xfcol_i  (one PSUM bank accum chain)
                h_ps = psha.tile([128, 1], f32, tag="ha")
                for gi, q in enumerate(qorder):
                    for j in range(4):
                        i = q * 4 + j
                        nc.tensor.matmul(
                            h_ps[:],
                            lhsT=rcols[:, i * 128:(i + 1) * 128],
                            rhs=xf_of(i),
                            start=(gi == 0 and j == 0),
                            stop=(gi == 3 and j == 3))
                h_sb = pp.tile([128, 1], bf16)
                nc.vector.tensor_copy(h_sb[:], h_ps[:])
                # a_ext = vwc_ext^T h : [1, 257] (col 256 = mean of a)
                a_ps = psha.tile([1, 257], f32, tag="ha")
                nc.tensor.matmul(a_ps[:], lhsT=h_sb[:], rhs=vwc[:],
                                 start=True, stop=True)
                a_sb = pp.tile([1, 257], f32)
                nc.scalar.activation(a_sb[:], a_ps[:], AF.Copy)
                # combined x_f (for o_x and the y elementwise product)
                xfcol = xfA
                nc.scalar.activation(xfcol[:, N_DVE_RELU:NT],
                                     xfB[:, N_DVE_RELU:NT], AF.Copy)
                nc.scalar.dma_start(o_x[:].rearrange("(i p) -> p i", p=128),
                                    xfcol[:])
                nc.scalar.dma_start(cf32[:], i_cf32[:])

            if stage >= 3:
                # ---- AllReduce #1 (a_ext) ----
                a_in = dram.tile([1, 257], f32)
                a_out = dram.tile([1, 257], f32)
                nc.sync.dma_start(a_in[:], a_sb[:])
                nc.gpsimd.collective_compute(
                    "AllReduce", ALU.add,
                    replica_groups=[list(range(N_CORES))],
                    ins=[a_in.opt()], outs=[a_out.opt()],
                )

            # rt = relu(V Dx^T) [T, n] bf16 (only needed for rho in phase B;
            # emitted here so it runs pre-AR on PE/Act/DVE slack)
            rt = pp.tile([128, NS], bf16)
            for q in range(NQ):
                rt_ps = psrt.tile([128, 512], f32, tag="rt")
                for c in range(2):
                    nc.tensor.matmul(
                        rt_ps[:],
                        lhsT=vts[:, c * 128:(c + 1) * 128],
                        rhs=dxts[:, q * 1024 + c * 512: q * 1024 + (c + 1) * 512],
                        start=(c == 0), stop=(c == 1))
                if q % 2 == 0:
                    nc.vector.tensor_scalar(rt[:, q * 512:(q + 1) * 512],
                                            rt_ps[:], 0.0, None, ALU.max)
                else:
                    nc.scalar.activation(rt[:, q * 512:(q + 1) * 512],
                                         rt_ps[:], AF.Relu)

            if stage >= 4:
                # ---- phase B: everything behind AllReduce #1 ----
                # acol [128,2]: a_d at [d%128, d//128]; mrow [1,1] = mean
                acol = pp.tile([128, 2], f32)
                mrow = pp.tile([1, 1], f32)
                nc.sync.dma_start(
                    acol[:], a_out[0:1, 0:256].rearrange("a (c p) -> (a p) c", p=128))
                nc.scalar.dma_start(mrow[:], a_out[0:1, 256:257])
                acolb = pp.tile([128, 2], bf16)
                nc.vector.tensor_copy(acolb[:], acol[:])

                # yca = Dy @ a  [128, 16] (in-order PE queue blocks on acolb)
                yca_ps = psb.tile([128, NT], f32, tag="b")
                for i in range(NT):
                    for c in range(2):
                        nc.tensor.matmul(
                            yca_ps[:, i:i + 1],
                            lhsT=dyts[:, c * NS + i * 128: c * NS + (i + 1) * 128],
                            rhs=acolb[:, c:c + 1],
                            start=(c == 0), stop=(c == 1))

                # sigma chain: std = sqrt((sum a^2 - 256 m^2)/255)
                sq = pp.tile([128, 2], f32)
                pssq = pp.tile([128, 1], f32)
                nc.vector.tensor_tensor_reduce(
                    out=sq[:], in0=acol[:], in1=acol[:], scale=1.0, scalar=0.0,
                    op0=ALU.mult, op1=ALU.add, accum_out=pssq[:])
                ssq_ps = psx.tile([1, 1], f32, tag="x")
                nc.tensor.matmul(ssq_ps[:], lhsT=pssq[:], rhs=ones_col[:],
                                 start=True, stop=True)
                msq = pp.tile([1, 1], f32)
                msq_o = pp.tile([1, 1], f32)
                nc.vector.tensor_tensor_reduce(
                    out=msq_o[:], in0=mrow[:], in1=mrow[:],
                    scale=-256.0 / 255.0, scalar=0.0,
                    op0=ALU.mult, op1=ALU.add, accum_out=msq[:])
                std = pp.tile([1, 1], f32)
                nc.scalar.activation(std[:], ssq_ps[:], AF.Sqrt,
                                     bias=msq[:], scale=1.0 / 255.0)
                scal2 = pp.tile([1, 2], f32)
                nc.vector.reciprocal(scal2[:, 0:1], std[:])
                nc.vector.tensor_mul(scal2[:, 1:2], mrow[:], scal2[:, 0:1])
                bc_ps = psx.tile([128, 2], f32, tag="x")
                nc.tensor.matmul(bc_ps[:], lhsT=ones_row[:], rhs=scal2[:],
                                 start=True, stop=True)
                cols = pp.tile([128, 2], f32)
                nc.vector.tensor_copy(cols[:], bc_ps[:])

                # y = relu(yca/s - (m/s) dy1) * x_f
                t1 = pp.tile([128, NT], f32)
                nc.vector.tensor_scalar(t1[:], yca_ps[:], cols[:, 0:1], None,
                                        ALU.mult)
                u2 = pp.tile([128, NT], f32)
                nc.gpsimd.tensor_scalar(u2[:], dy1col[:], cols[:, 1:2], None,
                                        ALU.mult)
                ysub = pp.tile([128, NT], f32)
                nc.vector.tensor_sub(ysub[:], t1[:], u2[:])
                ycr = pp.tile([128, NT], f32)
                nc.vector.tensor_scalar_max(ycr[:], ysub[:], 0.0)
                y = pp.tile([128, NT], bf16)
                nc.vector.tensor_mul(y[:], ycr[:], xfcol[:])
                nc.scalar.dma_start(o_y[:].rearrange("(i p) -> p i", p=128),
                                    y[:])

            if stage >= 5:
                # rho = vwp^T @ rt  [256, n]; runs on PE after yca (in-order),
                # i.e. inside the phase-B window, never inside an AR window.
                for dc in range(2):
                    rho_sb = wp.tile([128, NS], bf16, tag="rho_sb")
                    for q in range(NQ):
                        rho_ps = psrc.tile([128, 512], f32, tag="rc")
                        nc.tensor.matmul(rho_ps[:],
                                         lhsT=vwp[:, dc * 128:(dc + 1) * 128],
                                         rhs=rt[:, q * 512:(q + 1) * 512],
                                         start=True, stop=True)
                        dst = rho_sb[:, q * 512:(q + 1) * 512]
                        if dc == 0:
                            nc.scalar.activation(dst, rho_ps[:], AF.Copy)
                        else:
                            nc.vector.tensor_copy(dst, rho_ps[:])
                    nc.sync.dma_start(o_rho[dc * 128:(dc + 1) * 128, :],
                                      rho_sb[:])

            if stage >= 6:
                # vs_partial = y^T E^T : [1, 256] + pre-reduced mean col
                vs_ps = psb.tile([1, 256], f32, tag="b")
                for i in range(NT):
                    nc.tensor.matmul(vs_ps[:],
                                     lhsT=y[:, i:i + 1],
                                     rhs=ets[:, i * 256:(i + 1) * 256],
                                     start=(i == 0), stop=(i == NT - 1))
                vs_sb = pp.tile([1, 257], f32)
                vs_scr = pp.tile([1, 256], f32)
                nc.vector.tensor_copy(vs_sb[:, 0:256], vs_ps[:])
                nc.scalar.activation(vs_scr[:], vs_ps[:], AF.Copy,
                                     scale=1.0 / 256.0,
                                     accum_out=vs_sb[:, 256:257])

                # ---- AllReduce #2 (vs_ext) ----
                vs_in = dram.tile([1, 257], f32)
                vs_out = dram.tile([1, 257], f32)
                nc.sync.dma_start(vs_in[:], vs_sb[:])
                nc.gpsimd.collective_compute(
                    "AllReduce", ALU.add,
                    replica_groups=[list(range(N_CORES))],
                    ins=[vs_in.opt()], outs=[vs_out.opt()],
                )

            if stage >= 7:
                # ---- phase C: final layernorm of vs ----
                vrow = pp.tile([1, 257], f32)
                nc.sync.dma_start(vrow[:], vs_out[:])
                cen = pp.tile([1, 256], f32)
                nc.vector.tensor_scalar(cen[:], vrow[:, 0:256],
                                        vrow[:, 256:257], None, ALU.subtract)
                vsq = pp.tile([1, 256], f32)
                vssq = pp.tile([1, 1], f32)
                nc.vector.tensor_tensor_reduce(
                    out=vsq[:], in0=cen[:], in1=cen[:], scale=1.0 / 255.0,
                    scalar=0.0, op0=ALU.mult, op1=ALU.add, accum_out=vssq[:])
                vstd = pp.tile([1, 1], f32)
                nc.scalar.activation(vstd[:], vssq[:], AF.Sqrt)
                vinv = pp.tile([1, 1], f32)
                nc.vector.reciprocal(vinv[:], vstd[:])
                vsln = pp.tile([1, 256], f32)
                nc.vector.tensor_scalar(vsln[:], cen[:], vinv[:], None,
                                        ALU.mult)
                nc.sync.dma_start(o_vs[:].rearrange("(a b) -> a b", a=1),
                                  vsln[:])

    nc.finalize()
    return nc


def _host_prep(E, Dx, Dy, token_emb, tokens):
    import ml_dtypes
    bf = ml_dtypes.bfloat16

    E = np.asarray(E, dtype=np.float32)
    Dx = np.asarray(Dx, dtype=np.float32)
    Dy = np.asarray(Dy, dtype=np.float32)
    token_emb = np.asarray(token_emb, dtype=np.float32)
    tokens = np.asarray(tokens).astype(np.int64)

    v = np.ascontiguousarray(token_emb[tokens])          # [T, d]
    j = np.arange(T)
    w = (DECAY ** ((T - 1) - j)).astype(np.float32)
    w[T - 1] = 0.0
    wp = (DECAY ** (T - j)).astype(np.float32)
    u = np.triu(np.ones((T, T), dtype=np.float32))
    vwc = u @ (v * w[:, None])                           # [T, d]
    vwc_ext = np.concatenate([vwc, vwc.sum(1, keepdims=True) / 256.0], axis=1)
    vwp = u @ (v * wp[:, None])                          # [T, d]
    vts = np.concatenate([v[:, :128].T, v[:, 128:].T], axis=1)   # [128, 256]

    # vwp/vwc are [T, d] with T=128 partitions already
    cbf = np.ascontiguousarray(np.concatenate([vts, vwp, vwc_ext], axis=1))

    cf32 = np.zeros((128, 130), dtype=np.float32)
    cf32[:, 0] = 1.0
    cf32[0, 2:130] = 1.0

    in_maps = []
    for k in range(N_CORES):
        sl = slice(k * NS, (k + 1) * NS)
        dx_s = Dx[sl]                                    # [NS, 256]
        dy_s = Dy[sl]
        e_s = E[:, sl]                                   # [256, NS]
        dxq = []
        for q in range(4):
            rs = slice(q * 512, (q + 1) * 512)
            dxq.append(np.ascontiguousarray(np.concatenate(
                [dx_s[rs, :128].T, dx_s[rs, 128:].T], axis=1)).astype(bf))
        dyts = np.concatenate([dy_s[:, :128].T, dy_s[:, 128:].T], axis=1)
        ets = np.concatenate(
            [e_s[:, i * 128:(i + 1) * 128].T for i in range(NT)], axis=1)
        dy1 = dy_s.sum(axis=1)                           # [NS]
        dy1col = dy1.reshape(NT, 128).T                  # [128, 16]
        cbf_k = np.concatenate([cbf, dy1col], axis=1).astype(bf)
        in_maps.append({
            "dxq0": dxq[0], "dxq1": dxq[1], "dxq2": dxq[2], "dxq3": dxq[3],
            "dyts": np.ascontiguousarray(dyts).astype(bf),
            "ets": np.ascontiguousarray(ets).astype(bf),
            "cbf": np.ascontiguousarray(cbf_k),
            "cf32": cf32,
        })
    return in_maps


def kernel(E, Dx, Dy, token_emb, tokens, _trace=False):
    from concourse.bass_utils import run_bass_kernel_spmd

    key = ("nc", STAGE)
    if key not in _cache:
        _cache[key] = _build()
    nc = _cache[key]

    in_maps = _host_prep(E, Dx, Dy, token_emb, tokens)
    res = run_bass_kernel_spmd(nc, in_maps, core_ids=list(range(N_CORES)),
                               trace=_trace)
    _cache["last_result"] = res

    r = res.results
    x_full = np.concatenate([r[k]["out_x"] for k in range(N_CORES)])
    y_full = np.concatenate(
        [r[k]["out_y"].astype(np.float32) for k in range(N_CORES)])
    vs = r[0]["out_vs"]
    rho = np.concatenate(
        [r[k]["out_rho"].astype(np.float32) for k in range(N_CORES)], axis=1)
    return np.concatenate([x_full, y_full, vs, rho.ravel()]).astype(np.float32)


# revision 14
# speedup vs baseline: 1.1457x; 1.1274x over previous
"""Trainium2 Bass kernel for the token-scan problem (bf16 rewrite).

Math: the scan over T=128 tokens unrolls into dense matmuls:

  V     = token_emb[tokens]                    [T, d]
  R     = relu(Dx @ V^T)                       [n, T]  (rcols: [n,T], rt: [T,n])
  x_f   = R @ 1_T                              [n]     (accum_out of the relus)
  h     = R^T x_f                              [T]
  a*    = (U diag(w) V)^T h      (= vwc^T h)   [d]     -> AllReduce (with mean)
  yc    = Dy @ ln(a*) = (Dy a*)/s - (m/s)(Dy 1)
  y     = relu(yc) * x_f                       [n]
  vs    = ln(E @ y)                            [d]     -> AllReduce (with mean)
  rho   = (U diag(wp) V)^T @ R^T               [d, n]

Sharding: n split across 8 cores.  Two AllReduces of a 257-float vector
(payload + pre-reduced mean).  rho is computed between the two AllReduces
where the engines otherwise idle; each AllReduce window is kept free of
retiring instructions.
"""

import numpy as np

N, D, V_VOCAB, T = 16384, 256, 32000, 128
DECAY = 0.97
N_CORES = 8
NS = N // N_CORES           # 2048 rows per core
NT = NS // 128              # 16 tiles of 128
NQ = NS // 512              # 4 chunks of 512

N_DUMMY = 6                 # PE warm-up matmuls at t=0

_cache = {}
STAGE = 99


def _build():
    stage = STAGE
    import concourse.bacc as bacc
    import concourse.mybir as mybir
    import concourse.tile as tile

    f32 = mybir.dt.float32
    bf16 = mybir.dt.bfloat16
    AF = mybir.ActivationFunctionType
    ALU = mybir.AluOpType

    nc = bacc.Bacc("TRN2", target_bir_lowering=False, debug=False,
                   num_devices=N_CORES)

    # ---- dram inputs (per-core, host-prepped layouts) ----
    # dxq: 4 quarters, each [128, 1024] = both d-halves of 512 n-rows
    i_dxq = [nc.dram_tensor(f"dxq{q}", [128, 1024], bf16, kind="ExternalInput")
             for q in range(4)]
    i_dyts = nc.dram_tensor("dyts", [128, 2 * NS], bf16, kind="ExternalInput")
    i_ets = nc.dram_tensor("ets", [128, NT * 256], bf16, kind="ExternalInput")
    # cbf = [vts(256) | vwp(256) | vwc_ext(257) | dy1col(16)]
    i_cbf = nc.dram_tensor("cbf", [128, 785], bf16, kind="ExternalInput")
    # cf32: col0 = ones column; row0 cols 2..130 = ones row
    i_cf32 = nc.dram_tensor("cf32", [128, 130], f32, kind="ExternalInput")

    o_x = nc.dram_tensor("out_x", [NS], f32, kind="ExternalOutput")
    o_y = nc.dram_tensor("out_y", [NS], bf16, kind="ExternalOutput")
    o_vs = nc.dram_tensor("out_vs", [256], f32, kind="ExternalOutput")
    o_rho = nc.dram_tensor("out_rho", [256, NS], bf16, kind="ExternalOutput")

    with tile.TileContext(nc) as tc:
        with (
            tc.tile_pool(name="persist", bufs=1) as pp,
            tc.tile_pool(name="work", bufs=2) as wp,
            tc.tile_pool(name="psrc", bufs=3, space="PSUM") as psrc,
            tc.tile_pool(name="psrt", bufs=2, space="PSUM") as psrt,
            tc.tile_pool(name="psha", bufs=1, space="PSUM") as psha,
            tc.tile_pool(name="psb", bufs=1, space="PSUM") as psb,
            tc.tile_pool(name="psx", bufs=1, space="PSUM") as psx,
            tc.tile_pool(name="dram", bufs=1, space="DRAM") as dram,
        ):
            # ---------------- loads ----------------
            dxts = pp.tile([128, 4096], bf16)       # [q0d0 q0d1 q1d0 q1d1 ..]
            cbf = pp.tile([128, 785], bf16)
            cf32 = pp.tile([128, 130], f32)
            dyts = pp.tile([128, 2 * NS], bf16)
            ets = pp.tile([128, NT * 256], bf16)
            dm = pp.tile([128, 512], bf16)          # dummy-warmup operand

            # SP: cbf first (gates all matmuls), then dx quarters.
            # Pool: dx quarters + ets.  Act: kept clear for the early relus
            # (the framework's act-table load head-blocks it anyway).
            nc.sync.dma_start(cbf[:], i_cbf[:])
            nc.gpsimd.dma_start(dxts[:, 1024:2048], i_dxq[1][:])
            nc.sync.dma_start(dxts[:, 0:1024], i_dxq[0][:])
            nc.gpsimd.dma_start(dxts[:, 2048:3072], i_dxq[2][:])
            nc.sync.dma_start(dxts[:, 3072:4096], i_dxq[3][:])
            nc.vector.memset(dm[:], 0.0)
            # bulk loads for the post-AR phases; queues are free meanwhile
            nc.sync.dma_start(dyts[:], i_dyts[:])
            nc.gpsimd.dma_start(ets[:], i_ets[:])

            vts = cbf[:, 0:256]
            vwp = cbf[:, 256:512]
            vwc = cbf[:, 512:769]      # [128, 257] incl. mean column
            dy1col = cbf[:, 769:785]
            ones_col = cf32[:, 0:1]
            ones_row = cf32[0:1, 2:130]

            # ---------------- warm-ups ----------------
            # PE p-state ramp; plus a dummy Sqrt so the single act-table
            # load at t~1.5us picks a sqrt-capable set (they all contain
            # Relu/Copy/Square too) instead of a mid-kernel table swap.
            dmsq = pp.tile([1, 1], f32)
            nc.scalar.activation(dmsq[:], dm[0:1, 0:1], AF.Sqrt)
            for k in range(N_DUMMY):
                dm_ps = psrt.tile([128, 256], f32, tag="rt")
                nc.tensor.matmul(dm_ps[:], lhsT=dm[:, 0:128],
                                 rhs=dm[:, 0:256], start=True, stop=True)

            # ---------------- phase A: rcols -> h -> a* ----------------
            # rcols_i = relu(Dx_i V^T) [128n, 128T], fp32 in SBUF;
            # relu accum gives xfcol columns for free.  Per-engine xf
            # accumulator tiles avoid cross-engine WAW serialization.
            # DVE relu is ~2x cheaper per tile (258 vs 479 ns); give Act a
            # few early-group tiles only.
            ACT_TILES = {6, 7, 2, 3, 11}
            rcols = pp.tile([128, NT * 128], f32)
            xfA = pp.tile([128, NT], f32)
            xfB = pp.tile([128, NT], f32)
            xf_of = lambda i: (xfB if i in ACT_TILES else xfA)[:, i:i + 1]
            qorder = [1, 0, 2, 3]
            for gi, q in enumerate(qorder):
                rc_ps = psrc.tile([128, 512], f32, tag="rc")
                for j in range(4):
                    i = q * 4 + j
                    for c in range(2):
                        nc.tensor.matmul(
                            rc_ps[:, j * 128:(j + 1) * 128],
                            lhsT=dxts[:, q * 1024 + c * 512 + j * 128:
                                      q * 1024 + c * 512 + (j + 1) * 128],
                            rhs=vts[:, c * 128:(c + 1) * 128],
                            start=(c == 0), stop=(c == 1))
                for j in range(4):
                    i = q * 4 + j
                    src = rc_ps[:, j * 128:(j + 1) * 128]
                    dst = rcols[:, i * 128:(i + 1) * 128]
                    if i in ACT_TILES:
                        nc.scalar.activation(dst, src, AF.Relu,
                                             accum_out=xf_of(i))
                    else:
                        nc.vector.tensor_scalar(
                            dst, src, 0.0, None, ALU.max, op1=ALU.add,
                            accum_out=xf_of(i))

            if stage >= 2:
                # h = sum_i rcols_i^T xfcol_i  (one PSUM bank accum chain)
                h_ps = psha.tile([128, 1], f32, tag="ha")
                for gi, q in enumerate(qorder):
                    for j in range(4):
                        i = q * 4 + j
                        nc.tensor.matmul(
                            h_ps[:],
                            lhsT=rcols[:, i * 128:(i + 1) * 128],
                            rhs=xf_of(i),
                            start=(gi == 0 and j == 0),
                            stop=(gi == 3 and j == 3))
                h_sb = pp.tile([128, 1], bf16)
                nc.vector.tensor_copy(h_sb[:], h_ps[:])
                # a_ext = vwc_ext^T h : [1, 257] (col 256 = mean of a)
                a_ps = psha.tile([1, 257], f32, tag="ha")
                nc.tensor.matmul(a_ps[:], lhsT=h_sb[:], rhs=vwc[:],
                                 start=True, stop=True)
                a_sb = pp.tile([1, 257], f32)
                nc.scalar.activation(a_sb[:], a_ps[:], AF.Copy)
                # combined x_f (for o_x and the y elementwise product)
                xfcol = xfA
                nc.gpsimd.tensor_copy(xfcol[:, 2:4], xfB[:, 2:4])
                nc.gpsimd.tensor_copy(xfcol[:, 6:8], xfB[:, 6:8])
                nc.gpsimd.tensor_copy(xfcol[:, 11:12], xfB[:, 11:12])
                nc.scalar.dma_start(o_x[:].rearrange("(i p) -> p i", p=128),
                                    xfcol[:])
                nc.scalar.dma_start(cf32[:], i_cf32[:])

            if stage >= 3:
                # ---- AllReduce #1 (a_ext) ----
                a_in = dram.tile([1, 257], f32)
                a_out = dram.tile([1, 257], f32)
                nc.sync.dma_start(a_in[:], a_sb[:])
                nc.gpsimd.collective_compute(
                    "AllReduce", ALU.add,
                    replica_groups=[list(range(N_CORES))],
                    ins=[a_in.opt()], outs=[a_out.opt()],
                )

            # rt = relu(V Dx^T) [T, n] bf16 (only needed for rho in phase B;
            # emitted here so it runs pre-AR on PE/Act/DVE slack)
            rt = pp.tile([128, NS], bf16)
            for q in range(NQ):
                rt_ps = psrt.tile([128, 512], f32, tag="rt")
                for c in range(2):
                    nc.tensor.matmul(
                        rt_ps[:],
                        lhsT=vts[:, c * 128:(c + 1) * 128],
                        rhs=dxts[:, q * 1024 + c * 512: q * 1024 + (c + 1) * 512],
                        start=(c == 0), stop=(c == 1))
                if q % 2 == 0:
                    nc.vector.tensor_scalar(rt[:, q * 512:(q + 1) * 512],
                                            rt_ps[:], 0.0, None, ALU.max)
                else:
                    nc.scalar.activation(rt[:, q * 512:(q + 1) * 512],
                                         rt_ps[:], AF.Relu)

            if stage >= 4:
                # ---- phase B: everything behind AllReduce #1 ----
                # acol [128,2]: a_d at [d%128, d//128]; mrow [1,1] = mean
                acol = pp.tile([128, 2], f32)
                mrow = pp.tile([1, 1], f32)
                nc.sync.dma_start(
                    acol[:], a_out[0:1, 0:256].rearrange("a (c p) -> (a p) c", p=128))
                nc.scalar.dma_start(mrow[:], a_out[0:1, 256:257])
                acolb = pp.tile([128, 2], bf16)
                nc.vector.tensor_copy(acolb[:], acol[:])
                # rho inputs re-materialized with a data dependency on the
                # AllReduce result, so the scheduler cannot hoist the rho
                # chain into the AllReduce window (whose tail must stay
                # quiet for the collective to run at full HW speed).
                zcol = pp.tile([128, 1], f32)
                nc.gpsimd.tensor_scalar_mul(zcol[:], acol[:, 0:1], 0.0)
                vwp_w = pp.tile([128, 256], bf16)
                nc.gpsimd.tensor_scalar(vwp_w[:], vwp, zcol[:], None, ALU.add)

                # yca = Dy @ a  [128, 16] (in-order PE queue blocks on acolb)
                yca_ps = psb.tile([128, NT], f32, tag="b")
                for i in range(NT):
                    for c in range(2):
                        nc.tensor.matmul(
                            yca_ps[:, i:i + 1],
                            lhsT=dyts[:, c * NS + i * 128: c * NS + (i + 1) * 128],
                            rhs=acolb[:, c:c + 1],
                            start=(c == 0), stop=(c == 1))

                # sigma chain: std = sqrt((sum a^2 - 256 m^2)/255)
                sq = pp.tile([128, 2], f32)
                pssq = pp.tile([128, 1], f32)
                nc.vector.tensor_tensor_reduce(
                    out=sq[:], in0=acol[:], in1=acol[:], scale=1.0, scalar=0.0,
                    op0=ALU.mult, op1=ALU.add, accum_out=pssq[:])
                ssq_ps = psx.tile([1, 1], f32, tag="x")
                nc.tensor.matmul(ssq_ps[:], lhsT=pssq[:], rhs=ones_col[:],
                                 start=True, stop=True)
                msq = pp.tile([1, 1], f32)
                msq_o = pp.tile([1, 1], f32)
                nc.vector.tensor_tensor_reduce(
                    out=msq_o[:], in0=mrow[:], in1=mrow[:],
                    scale=-256.0 / 255.0, scalar=0.0,
                    op0=ALU.mult, op1=ALU.add, accum_out=msq[:])
                std = pp.tile([1, 1], f32)
                nc.scalar.activation(std[:], ssq_ps[:], AF.Sqrt,
                                     bias=msq[:], scale=1.0 / 255.0)
                scal2 = pp.tile([1, 2], f32)
                nc.vector.reciprocal(scal2[:, 0:1], std[:])
                nc.vector.tensor_mul(scal2[:, 1:2], mrow[:], scal2[:, 0:1])
                bc_ps = psx.tile([128, 2], f32, tag="x")
                nc.tensor.matmul(bc_ps[:], lhsT=ones_row[:], rhs=scal2[:],
                                 start=True, stop=True)
                cols = pp.tile([128, 2], f32)
                nc.vector.tensor_copy(cols[:], bc_ps[:])

                # y = relu(yca/s - (m/s) dy1) * x_f
                t1 = pp.tile([128, NT], f32)
                nc.vector.tensor_scalar(t1[:], yca_ps[:], cols[:, 0:1], None,
                                        ALU.mult)
                u2 = pp.tile([128, NT], f32)
                nc.gpsimd.tensor_scalar(u2[:], dy1col[:], cols[:, 1:2], None,
                                        ALU.mult)
                ysub = pp.tile([128, NT], f32)
                nc.vector.tensor_sub(ysub[:], t1[:], u2[:])
                ycr = pp.tile([128, NT], f32)
                nc.vector.tensor_scalar_max(ycr[:], ysub[:], 0.0)
                y = pp.tile([128, NT], bf16)
                nc.vector.tensor_mul(y[:], ycr[:], xfcol[:])
                nc.scalar.dma_start(o_y[:].rearrange("(i p) -> p i", p=128),
                                    y[:])

            if stage >= 5:
                # rho = vwp^T @ rt  [256, n]; runs on PE after yca (in-order),
                # i.e. inside the phase-B window, never inside an AR window.
                for dc in range(2):
                    rho_sb = wp.tile([128, NS], bf16, tag="rho_sb")
                    for q in range(NQ):
                        rho_ps = psrc.tile([128, 512], f32, tag="rc")
                        nc.tensor.matmul(rho_ps[:],
                                         lhsT=vwp_w[:, dc * 128:(dc + 1) * 128],
                                         rhs=rt[:, q * 512:(q + 1) * 512],
                                         start=True, stop=True)
                        dst = rho_sb[:, q * 512:(q + 1) * 512]
                        if dc == 0:
                            nc.scalar.activation(dst, rho_ps[:], AF.Copy)
                        else:
                            nc.vector.tensor_copy(dst, rho_ps[:])
                    nc.sync.dma_start(o_rho[dc * 128:(dc + 1) * 128, :],
                                      rho_sb[:])

            if stage >= 6:
                # vs_partial = y^T E^T : [1, 256] + pre-reduced mean col
                vs_ps = psb.tile([1, 256], f32, tag="b")
                for i in range(NT):
                    nc.tensor.matmul(vs_ps[:],
                                     lhsT=y[:, i:i + 1],
                                     rhs=ets[:, i * 256:(i + 1) * 256],
                                     start=(i == 0), stop=(i == NT - 1))
                vs_sb = pp.tile([1, 257], f32)
                vs_scr = pp.tile([1, 256], f32)
                nc.vector.tensor_copy(vs_sb[:, 0:256], vs_ps[:])
                nc.scalar.activation(vs_scr[:], vs_ps[:], AF.Copy,
                                     scale=1.0 / 256.0,
                                     accum_out=vs_sb[:, 256:257])

                # ---- AllReduce #2 (vs_ext) ----
                vs_in = dram.tile([1, 257], f32)
                vs_out = dram.tile([1, 257], f32)
                nc.sync.dma_start(vs_in[:], vs_sb[:])
                nc.gpsimd.collective_compute(
                    "AllReduce", ALU.add,
                    replica_groups=[list(range(N_CORES))],
                    ins=[vs_in.opt()], outs=[vs_out.opt()],
                )

            if stage >= 7:
                # ---- phase C: final layernorm of vs ----
                vrow = pp.tile([1, 257], f32)
                nc.sync.dma_start(vrow[:], vs_out[:])
                cen = pp.tile([1, 256], f32)
                nc.vector.tensor_scalar(cen[:], vrow[:, 0:256],
                                        vrow[:, 256:257], None, ALU.subtract)
                vsq = pp.tile([1, 256], f32)
                vssq = pp.tile([1, 1], f32)
                nc.vector.tensor_tensor_reduce(
                    out=vsq[:], in0=cen[:], in1=cen[:], scale=1.0 / 255.0,
                    scalar=0.0, op0=ALU.mult, op1=ALU.add, accum_out=vssq[:])
                vstd = pp.tile([1, 1], f32)
                nc.scalar.activation(vstd[:], vssq[:], AF.Sqrt)
                vinv = pp.tile([1, 1], f32)
                nc.vector.reciprocal(vinv[:], vstd[:])
                vsln = pp.tile([1, 256], f32)
                nc.vector.tensor_scalar(vsln[:], cen[:], vinv[:], None,
                                        ALU.mult)
                nc.sync.dma_start(o_vs[:].rearrange("(a b) -> a b", a=1),
                                  vsln[:])

    nc.finalize()
    return nc


def _host_prep(E, Dx, Dy, token_emb, tokens):
    import ml_dtypes
    bf = ml_dtypes.bfloat16

    E = np.asarray(E, dtype=np.float32)
    Dx = np.asarray(Dx, dtype=np.float32)
    Dy = np.asarray(Dy, dtype=np.float32)
    token_emb = np.asarray(token_emb, dtype=np.float32)
    tokens = np.asarray(tokens).astype(np.int64)

    v = np.ascontiguousarray(token_emb[tokens])          # [T, d]
    j = np.arange(T)
    w = (DECAY ** ((T - 1) - j)).astype(np.float32)
    w[T - 1] = 0.0
    wp = (DECAY ** (T - j)).astype(np.float32)
    u = np.triu(np.ones((T, T), dtype=np.float32))
    vwc = u @ (v * w[:, None])                           # [T, d]
    vwc_ext = np.concatenate([vwc, vwc.sum(1, keepdims=True) / 256.0], axis=1)
    vwp = u @ (v * wp[:, None])                          # [T, d]
    vts = np.concatenate([v[:, :128].T, v[:, 128:].T], axis=1)   # [128, 256]

    # vwp/vwc are [T, d] with T=128 partitions already
    cbf = np.ascontiguousarray(np.concatenate([vts, vwp, vwc_ext], axis=1))

    cf32 = np.zeros((128, 130), dtype=np.float32)
    cf32[:, 0] = 1.0
    cf32[0, 2:130] = 1.0

    in_maps = []
    for k in range(N_CORES):
        sl = slice(k * NS, (k + 1) * NS)
        dx_s = Dx[sl]                                    # [NS, 256]
        dy_s = Dy[sl]
        e_s = E[:, sl]                                   # [256, NS]
        dxq = []
        for q in range(4):
            rs = slice(q * 512, (q + 1) * 512)
            dxq.append(np.ascontiguousarray(np.concatenate(
                [dx_s[rs, :128].T, dx_s[rs, 128:].T], axis=1)).astype(bf))
        dyts = np.concatenate([dy_s[:, :128].T, dy_s[:, 128:].T], axis=1)
        ets = np.concatenate(
            [e_s[:, i * 128:(i + 1) * 128].T for i in range(NT)], axis=1)
        dy1 = dy_s.sum(axis=1)                           # [NS]
        dy1col = dy1.reshape(NT, 128).T                  # [128, 16]
        cbf_k = np.concatenate([cbf, dy1col], axis=1).astype(bf)
        in_maps.append({
            "dxq0": dxq[0], "dxq1": dxq[1], "dxq2": dxq[2], "dxq3": dxq[3],
            "dyts": np.ascontiguousarray(dyts).astype(bf),
            "ets": np.ascontiguousarray(ets).astype(bf),
            "cbf": np.ascontiguousarray(cbf_k),
            "cf32": cf32,
        })
    return in_maps


def kernel(E, Dx, Dy, token_emb, tokens, _trace=False):
    from concourse.bass_utils import run_bass_kernel_spmd

    key = ("nc", STAGE)
    if key not in _cache:
        _cache[key] = _build()
    nc = _cache[key]

    in_maps = _host_prep(E, Dx, Dy, token_emb, tokens)
    res = run_bass_kernel_spmd(nc, in_maps, core_ids=list(range(N_CORES)),
                               trace=_trace)
    _cache["last_result"] = res

    r = res.results
    x_full = np.concatenate([r[k]["out_x"] for k in range(N_CORES)])
    y_full = np.concatenate(
        [r[k]["out_y"].astype(np.float32) for k in range(N_CORES)])
    vs = r[0]["out_vs"]
    rho = np.concatenate(
        [r[k]["out_rho"].astype(np.float32) for k in range(N_CORES)], axis=1)
    return np.concatenate([x_full, y_full, vs, rho.ravel()]).astype(np.float32)


# revision 24
# speedup vs baseline: 1.1686x; 1.0200x over previous
"""Trainium2 Bass kernel for the token-scan problem (bf16 rewrite).

Math: the scan over T=128 tokens unrolls into dense matmuls:

  V     = token_emb[tokens]                    [T, d]
  R     = relu(Dx @ V^T)                       [n, T]  (rcols: [n,T], rt: [T,n])
  x_f   = R @ 1_T                              [n]     (accum_out of the relus)
  h     = R^T x_f                              [T]
  a*    = (U diag(w) V)^T h      (= vwc^T h)   [d]     -> AllReduce (with mean)
  yc    = Dy @ ln(a*) = (Dy a*)/s - (m/s)(Dy 1)
  y     = relu(yc) * x_f                       [n]
  vs    = ln(E @ y)                            [d]     -> AllReduce (with mean)
  rho   = (U diag(wp) V)^T @ R^T               [d, n]

Sharding: n split across 8 cores.  Two AllReduces of a 257-float vector
(payload + pre-reduced mean).  rho is computed between the two AllReduces
where the engines otherwise idle; each AllReduce window is kept free of
retiring instructions.
"""

import numpy as np

N, D, V_VOCAB, T = 16384, 256, 32000, 128
DECAY = 0.97
N_CORES = 8
NS = N // N_CORES           # 2048 rows per core
NT = NS // 128              # 16 tiles of 128
NQ = NS // 512              # 4 chunks of 512

N_DUMMY = 6                 # PE warm-up matmuls at t=0

_cache = {}
STAGE = 99


def _build():
    stage = STAGE
    import concourse.bacc as bacc
    import concourse.mybir as mybir
    import concourse.tile as tile

    f32 = mybir.dt.float32
    bf16 = mybir.dt.bfloat16
    AF = mybir.ActivationFunctionType
    ALU = mybir.AluOpType

    nc = bacc.Bacc("TRN2", target_bir_lowering=False, debug=False,
                   num_devices=N_CORES)

    # ---- dram inputs (per-core, host-prepped layouts) ----
    # dxq: 4 quarters, each [128, 1024] = both d-halves of 512 n-rows
    i_dxq = [nc.dram_tensor(f"dxq{q}", [128, 1024], bf16, kind="ExternalInput")
             for q in range(4)]
    i_dyts = nc.dram_tensor("dyts", [128, 2 * NS], bf16, kind="ExternalInput")
    i_ets = nc.dram_tensor("ets", [128, NT * 256], bf16, kind="ExternalInput")
    # cbf = [vts(256) | vwp(256) | vwc_ext(257)]
    i_cbf = nc.dram_tensor("cbf", [128, 769], bf16, kind="ExternalInput")
    # cf32: col0 = ones column; row0 cols 2..130 = ones row
    i_cf32 = nc.dram_tensor("cf32", [128, 130], f32, kind="ExternalInput")

    o_x = nc.dram_tensor("out_x", [NS], f32, kind="ExternalOutput")
    o_y = nc.dram_tensor("out_y", [NS], bf16, kind="ExternalOutput")
    o_vs = nc.dram_tensor("out_vs", [256], f32, kind="ExternalOutput")
    o_s = nc.dram_tensor("out_s", [1], f32, kind="ExternalOutput")
    o_rho = nc.dram_tensor("out_rho", [256, NS], bf16, kind="ExternalOutput")

    with tile.TileContext(nc) as tc:
        with (
            tc.tile_pool(name="persist", bufs=1) as pp,
            tc.tile_pool(name="work", bufs=2) as wp,
            tc.tile_pool(name="psrc", bufs=3, space="PSUM") as psrc,
            tc.tile_pool(name="psrt", bufs=2, space="PSUM") as psrt,
            tc.tile_pool(name="psha", bufs=1, space="PSUM") as psha,
            tc.tile_pool(name="psb", bufs=1, space="PSUM") as psb,
            tc.tile_pool(name="psx", bufs=1, space="PSUM") as psx,
            tc.tile_pool(name="dram", bufs=1, space="DRAM") as dram,
        ):
            # ---------------- loads ----------------
            dxts = pp.tile([128, 4096], bf16)       # [q0d0 q0d1 q1d0 q1d1 ..]
            cbf = pp.tile([128, 769], bf16)
            cf32 = pp.tile([128, 130], f32)
            dyts = pp.tile([128, 2 * NS], bf16)
            ets = pp.tile([128, NT * 256], bf16)
            dm = pp.tile([128, 512], bf16)          # dummy-warmup operand

            # SP: cbf first (gates all matmuls), then dx quarters.
            # Pool: dx quarters + ets.  Act: kept clear for the early relus
            # (the framework's act-table load head-blocks it anyway).
            nc.sync.dma_start(cbf[:], i_cbf[:])
            nc.gpsimd.dma_start(dxts[:, 1024:2048], i_dxq[1][:])
            nc.sync.dma_start(dxts[:, 0:1024], i_dxq[0][:])
            nc.gpsimd.dma_start(dxts[:, 2048:3072], i_dxq[2][:])
            nc.sync.dma_start(dxts[:, 3072:4096], i_dxq[3][:])
            nc.vector.memset(dm[:], 0.0)
            # bulk loads for the post-AR phases; queues are free meanwhile
            nc.sync.dma_start(dyts[:], i_dyts[:])
            nc.gpsimd.dma_start(ets[:], i_ets[:])

            vts = cbf[:, 0:256]
            vwp = cbf[:, 256:512]
            vwc = cbf[:, 512:769]      # [128, 257] incl. mean column
            ones_col = cf32[:, 0:1]
            ones_row = cf32[0:1, 2:130]

            # ---------------- warm-ups ----------------
            # PE p-state ramp; plus a dummy Sqrt so the single act-table
            # load at t~1.5us picks a sqrt-capable set (they all contain
            # Relu/Copy/Square too) instead of a mid-kernel table swap.
            dmsq = pp.tile([1, 1], f32)
            nc.scalar.activation(dmsq[:], dm[0:1, 0:1], AF.Sqrt)
            for k in range(N_DUMMY):
                dm_ps = psrt.tile([128, 256], f32, tag="rt")
                nc.tensor.matmul(dm_ps[:], lhsT=dm[:, 0:128],
                                 rhs=dm[:, 0:256], start=True, stop=True)

            # ---------------- phase A: rcols -> h -> a* ----------------
            # rcols_i = relu(Dx_i V^T) [128n, 128T], fp32 in SBUF;
            # relu accum gives xfcol columns for free.  Per-engine xf
            # accumulator tiles avoid cross-engine WAW serialization.
            # DVE relu is ~2x cheaper per tile (258 vs 479 ns); give Act a
            # few early-group tiles only.
            ACT_TILES = {6, 7, 2, 3, 11}
            rcols = pp.tile([128, NT * 128], f32)
            xfA = pp.tile([128, NT], f32)
            xfB = pp.tile([128, NT], f32)
            xf_of = lambda i: (xfB if i in ACT_TILES else xfA)[:, i:i + 1]
            qorder = [1, 0, 2, 3]
            for gi, q in enumerate(qorder):
                rc_ps = psrc.tile([128, 512], f32, tag="rc")
                for j in range(4):
                    i = q * 4 + j
                    for c in range(2):
                        nc.tensor.matmul(
                            rc_ps[:, j * 128:(j + 1) * 128],
                            lhsT=dxts[:, q * 1024 + c * 512 + j * 128:
                                      q * 1024 + c * 512 + (j + 1) * 128],
                            rhs=vts[:, c * 128:(c + 1) * 128],
                            start=(c == 0), stop=(c == 1))
                for j in range(4):
                    i = q * 4 + j
                    src = rc_ps[:, j * 128:(j + 1) * 128]
                    dst = rcols[:, i * 128:(i + 1) * 128]
                    if i in ACT_TILES:
                        nc.scalar.activation(dst, src, AF.Relu,
                                             accum_out=xf_of(i))
                    else:
                        nc.vector.tensor_scalar(
                            dst, src, 0.0, None, ALU.max, op1=ALU.add,
                            accum_out=xf_of(i))

            if stage >= 2:
                # h = sum_i rcols_i^T xfcol_i  (one PSUM bank accum chain)
                h_ps = psha.tile([128, 1], f32, tag="ha")
                for gi, q in enumerate(qorder):
                    for j in range(4):
                        i = q * 4 + j
                        nc.tensor.matmul(
                            h_ps[:],
                            lhsT=rcols[:, i * 128:(i + 1) * 128],
                            rhs=xf_of(i),
                            start=(gi == 0 and j == 0),
                            stop=(gi == 3 and j == 3))
                h_sb = pp.tile([128, 1], bf16)
                nc.vector.tensor_copy(h_sb[:], h_ps[:])
                # zero column pinned behind h: rt relus take it as operand so
                # the scheduler cannot slot them into the h-critical chain
                zA = pp.tile([128, 1], f32)
                nc.vector.tensor_scalar_mul(zA[:], h_sb[:], 0.0)
                # a_ext = vwc_ext^T h : [1, 257] (col 256 = mean of a)
                a_ps = psha.tile([1, 257], f32, tag="ha")
                nc.tensor.matmul(a_ps[:], lhsT=h_sb[:], rhs=vwc[:],
                                 start=True, stop=True)
                a_sb = pp.tile([1, 257], f32)
                nc.scalar.activation(a_sb[:], a_ps[:], AF.Copy)
                # combined x_f (for o_x and the y elementwise product)
                xfcol = xfA
                nc.gpsimd.tensor_copy(xfcol[:, 2:4], xfB[:, 2:4])
                nc.gpsimd.tensor_copy(xfcol[:, 6:8], xfB[:, 6:8])
                nc.gpsimd.tensor_copy(xfcol[:, 11:12], xfB[:, 11:12])
                nc.scalar.dma_start(o_x[:].rearrange("(i p) -> p i", p=128),
                                    xfcol[:])
                nc.scalar.dma_start(cf32[:], i_cf32[:])

            if stage >= 3:
                # ---- AllReduce #1 (a_ext) ----
                a_in = dram.tile([1, 257], f32)
                a_out = dram.tile([1, 257], f32)
                nc.sync.dma_start(a_in[:], a_sb[:])
                nc.gpsimd.collective_compute(
                    "AllReduce", ALU.add,
                    replica_groups=[list(range(N_CORES))],
                    ins=[a_in.opt()], outs=[a_out.opt()],
                )

            # rt = relu(V Dx^T) [T, n] bf16 (only needed for rho in phase B;
            # relus read zA so they run after the h chain, pre-AR slack)
            rt = pp.tile([128, NS], bf16)
            for q in range(NQ):
                rt_ps = psrt.tile([128, 512], f32, tag="rt")
                for c in range(2):
                    nc.tensor.matmul(
                        rt_ps[:],
                        lhsT=vts[:, c * 128:(c + 1) * 128],
                        rhs=dxts[:, q * 1024 + c * 512: q * 1024 + (c + 1) * 512],
                        start=(c == 0), stop=(c == 1))
                if q % 2 == 0:
                    nc.vector.tensor_scalar(rt[:, q * 512:(q + 1) * 512],
                                            rt_ps[:], zA[:], None, ALU.max)
                else:
                    nc.scalar.activation(rt[:, q * 512:(q + 1) * 512],
                                         rt_ps[:], AF.Relu, bias=zA[:])

            if stage >= 4:
                # ---- phase B: everything behind AllReduce #1 ----
                # Since relu(z/s) = relu(z)/s and the final layernorm is
                # scale-invariant, the 1/std factor of ln(a*) cancels in
                # out_vs; y is computed unscaled and rescaled on the host
                # via the exported out_s.  Dy@ln(a) then reduces to
                # Dy@(a - m), with m folded into the AllReduce payload.
                acol = pp.tile([128, 2], f32)
                mrow = pp.tile([1, 1], f32)
                nc.sync.dma_start(
                    acol[:], a_out[0:1, 0:256].rearrange("a (c p) -> (a p) c", p=128))
                nc.scalar.dma_start(mrow[:], a_out[0:1, 256:257])
                # broadcast m across partitions via a rank-1 matmul
                mb_ps = psx.tile([128, 1], f32, tag="x")
                nc.tensor.matmul(mb_ps[:], lhsT=ones_row[:], rhs=mrow[:],
                                 start=True, stop=True)
                m_col = pp.tile([128, 1], f32)
                nc.vector.tensor_copy(m_col[:], mb_ps[:])
                cenb = pp.tile([128, 2], bf16)
                nc.vector.tensor_scalar(cenb[:], acol[:], m_col[:], None,
                                        ALU.subtract)
                # rho inputs re-materialized with a data dependency on the
                # AllReduce result, so the scheduler cannot hoist the rho
                # chain into the AllReduce window (whose tail must stay
                # quiet for the collective to run at full HW speed).
                zcol = pp.tile([128, 1], f32)
                nc.gpsimd.tensor_scalar_mul(zcol[:], acol[:, 0:1], 0.0)
                vwp_w = pp.tile([128, 256], bf16)
                nc.gpsimd.tensor_scalar(vwp_w[:], vwp, zcol[:], None, ALU.add)

                # yc~ = Dy @ (a - m)  [128, 16]
                yca_ps = psb.tile([128, NT], f32, tag="b")
                for i in range(NT):
                    for c in range(2):
                        nc.tensor.matmul(
                            yca_ps[:, i:i + 1],
                            lhsT=dyts[:, c * NS + i * 128: c * NS + (i + 1) * 128],
                            rhs=cenb[:, c:c + 1],
                            start=(c == 0), stop=(c == 1))
                ycr = pp.tile([128, NT], f32)
                nc.vector.tensor_scalar_max(ycr[:], yca_ps[:], 0.0)
                y = pp.tile([128, NT], bf16)
                nc.vector.tensor_mul(y[:], ycr[:], xfcol[:])
                nc.scalar.dma_start(o_y[:].rearrange("(i p) -> p i", p=128),
                                    y[:])

                # sigma side-chain (only for the host rescale of y):
                # std = sqrt((sum a^2 - 256 m^2)/255), exported as out_s
                sq = pp.tile([128, 2], f32)
                pssq = pp.tile([128, 1], f32)
                nc.vector.tensor_tensor_reduce(
                    out=sq[:], in0=acol[:], in1=acol[:], scale=1.0, scalar=0.0,
                    op0=ALU.mult, op1=ALU.add, accum_out=pssq[:])
                ssq_ps = psx.tile([1, 1], f32, tag="x")
                nc.tensor.matmul(ssq_ps[:], lhsT=pssq[:], rhs=ones_col[:],
                                 start=True, stop=True)
                msq = pp.tile([1, 1], f32)
                msq_o = pp.tile([1, 1], f32)
                nc.vector.tensor_tensor_reduce(
                    out=msq_o[:], in0=mrow[:], in1=mrow[:],
                    scale=-256.0 / 255.0, scalar=0.0,
                    op0=ALU.mult, op1=ALU.add, accum_out=msq[:])
                std = pp.tile([1, 1], f32)
                nc.scalar.activation(std[:], ssq_ps[:], AF.Sqrt,
                                     bias=msq[:], scale=1.0 / 255.0)
                nc.scalar.dma_start(o_s[:].rearrange("(a b) -> a b", a=1),
                                    std[:])

            if stage >= 5:
                # rho = vwp^T @ rt  [256, n]; runs on PE after yca (in-order),
                # i.e. inside the phase-B window, never inside an AR window.
                for dc in range(2):
                    rho_sb = wp.tile([128, NS], bf16, tag="rho_sb")
                    for q in range(NQ):
                        rho_ps = psrc.tile([128, 512], f32, tag="rc")
                        nc.tensor.matmul(rho_ps[:],
                                         lhsT=vwp_w[:, dc * 128:(dc + 1) * 128],
                                         rhs=rt[:, q * 512:(q + 1) * 512],
                                         start=True, stop=True)
                        dst = rho_sb[:, q * 512:(q + 1) * 512]
                        if dc == 0:
                            nc.scalar.activation(dst, rho_ps[:], AF.Copy)
                        else:
                            nc.vector.tensor_copy(dst, rho_ps[:])
                    nc.sync.dma_start(o_rho[dc * 128:(dc + 1) * 128, :],
                                      rho_sb[:])

            if stage >= 6:
                # vs_partial = y^T E^T : [1, 256] + pre-reduced mean col
                vs_ps = psb.tile([1, 256], f32, tag="b")
                for i in range(NT):
                    nc.tensor.matmul(vs_ps[:],
                                     lhsT=y[:, i:i + 1],
                                     rhs=ets[:, i * 256:(i + 1) * 256],
                                     start=(i == 0), stop=(i == NT - 1))
                # separate tiles per engine (same-tile writes from two
                # engines serialize via WAW otherwise)
                vs_sb0 = pp.tile([1, 256], f32)
                vs_m = pp.tile([1, 1], f32)
                vs_scr = pp.tile([1, 256], f32)
                nc.vector.tensor_copy(vs_sb0[:], vs_ps[:])
                nc.scalar.activation(vs_scr[:], vs_ps[:], AF.Copy,
                                     scale=1.0 / 256.0, accum_out=vs_m[:])

                # ---- AllReduce #2 (vs_ext) ----
                vs_in = dram.tile([1, 257], f32)
                vs_out = dram.tile([1, 257], f32)
                nc.sync.dma_start(vs_in[0:1, 0:256], vs_sb0[:])
                nc.scalar.dma_start(vs_in[0:1, 256:257], vs_m[:])
                nc.gpsimd.collective_compute(
                    "AllReduce", ALU.add,
                    replica_groups=[list(range(N_CORES))],
                    ins=[vs_in.opt()], outs=[vs_out.opt()],
                )

            if stage >= 7:
                # ---- phase C: final layernorm of vs ----
                vrow = pp.tile([1, 257], f32)
                nc.sync.dma_start(vrow[:], vs_out[:])
                cen = pp.tile([1, 256], f32)
                nc.vector.tensor_scalar(cen[:], vrow[:, 0:256],
                                        vrow[:, 256:257], None, ALU.subtract)
                vsq = pp.tile([1, 256], f32)
                vssq = pp.tile([1, 1], f32)
                nc.vector.tensor_tensor_reduce(
                    out=vsq[:], in0=cen[:], in1=cen[:], scale=1.0 / 255.0,
                    scalar=0.0, op0=ALU.mult, op1=ALU.add, accum_out=vssq[:])
                vstd = pp.tile([1, 1], f32)
                nc.scalar.activation(vstd[:], vssq[:], AF.Sqrt)
                vinv = pp.tile([1, 1], f32)
                nc.vector.reciprocal(vinv[:], vstd[:])
                vsln = pp.tile([1, 256], f32)
                nc.vector.tensor_scalar(vsln[:], cen[:], vinv[:], None,
                                        ALU.mult)
                nc.sync.dma_start(o_vs[:].rearrange("(a b) -> a b", a=1),
                                  vsln[:])

    nc.finalize()
    return nc


def _host_prep(E, Dx, Dy, token_emb, tokens):
    import ml_dtypes
    bf = ml_dtypes.bfloat16

    E = np.asarray(E, dtype=np.float32)
    Dx = np.asarray(Dx, dtype=np.float32)
    Dy = np.asarray(Dy, dtype=np.float32)
    token_emb = np.asarray(token_emb, dtype=np.float32)
    tokens = np.asarray(tokens).astype(np.int64)

    v = np.ascontiguousarray(token_emb[tokens])          # [T, d]
    j = np.arange(T)
    w = (DECAY ** ((T - 1) - j)).astype(np.float32)
    w[T - 1] = 0.0
    wp = (DECAY ** (T - j)).astype(np.float32)
    u = np.triu(np.ones((T, T), dtype=np.float32))
    vwc = u @ (v * w[:, None])                           # [T, d]
    vwc_ext = np.concatenate([vwc, vwc.sum(1, keepdims=True) / 256.0], axis=1)
    vwp = u @ (v * wp[:, None])                          # [T, d]
    vts = np.concatenate([v[:, :128].T, v[:, 128:].T], axis=1)   # [128, 256]

    # vwp/vwc are [T, d] with T=128 partitions already
    cbf = np.ascontiguousarray(np.concatenate([vts, vwp, vwc_ext], axis=1))

    cf32 = np.zeros((128, 130), dtype=np.float32)
    cf32[:, 0] = 1.0
    cf32[0, 2:130] = 1.0

    in_maps = []
    for k in range(N_CORES):
        sl = slice(k * NS, (k + 1) * NS)
        dx_s = Dx[sl]                                    # [NS, 256]
        dy_s = Dy[sl]
        e_s = E[:, sl]                                   # [256, NS]
        dxq = []
        for q in range(4):
            rs = slice(q * 512, (q + 1) * 512)
            dxq.append(np.ascontiguousarray(np.concatenate(
                [dx_s[rs, :128].T, dx_s[rs, 128:].T], axis=1)).astype(bf))
        dyts = np.concatenate([dy_s[:, :128].T, dy_s[:, 128:].T], axis=1)
        ets = np.concatenate(
            [e_s[:, i * 128:(i + 1) * 128].T for i in range(NT)], axis=1)
        in_maps.append({
            "dxq0": dxq[0], "dxq1": dxq[1], "dxq2": dxq[2], "dxq3": dxq[3],
            "dyts": np.ascontiguousarray(dyts).astype(bf),
            "ets": np.ascontiguousarray(ets).astype(bf),
            "cbf": np.ascontiguousarray(cbf.astype(bf)),
            "cf32": cf32,
        })
    return in_maps


def kernel(E, Dx, Dy, token_emb, tokens, _trace=False):
    from concourse.bass_utils import run_bass_kernel_spmd

    key = ("nc", STAGE)
    if key not in _cache:
        _cache[key] = _build()
    nc = _cache[key]

    in_maps = _host_prep(E, Dx, Dy, token_emb, tokens)
    res = run_bass_kernel_spmd(nc, in_maps, core_ids=list(range(N_CORES)),
                               trace=_trace)
    _cache["last_result"] = res

    r = res.results
    x_full = np.concatenate([r[k]["out_x"] for k in range(N_CORES)])
    # out_y is the unscaled relu(Dy(a-m))*x_f; divide by the exported std
    y_full = np.concatenate(
        [r[k]["out_y"].astype(np.float32) / r[k]["out_s"][0].astype(np.float32)
         for k in range(N_CORES)])
    vs = r[0]["out_vs"]
    rho = np.concatenate(
        [r[k]["out_rho"].astype(np.float32) for k in range(N_CORES)], axis=1)
    return np.concatenate([x_full, y_full, vs, rho.ravel()]).astype(np.float32)


# revision 26
# speedup vs baseline: 1.1980x; 1.0252x over previous
"""Trainium2 Bass kernel for the token-scan problem (bf16 rewrite).

Math: the scan over T=128 tokens unrolls into dense matmuls:

  V     = token_emb[tokens]                    [T, d]
  R     = relu(Dx @ V^T)                       [n, T]  (rcols: [n,T], rt: [T,n])
  x_f   = R @ 1_T                              [n]     (accum_out of the relus)
  h     = R^T x_f                              [T]
  a*    = (U diag(w) V)^T h      (= vwc^T h)   [d]     -> AllReduce (with mean)
  yc    = Dy @ ln(a*) = (Dy a*)/s - (m/s)(Dy 1)
  y     = relu(yc) * x_f                       [n]
  vs    = ln(E @ y)                            [d]     -> AllReduce (with mean)
  rho   = (U diag(wp) V)^T @ R^T               [d, n]

Sharding: n split across 8 cores.  Two AllReduces of a 257-float vector
(payload + pre-reduced mean).  rho is computed between the two AllReduces
where the engines otherwise idle; each AllReduce window is kept free of
retiring instructions.
"""

import numpy as np

N, D, V_VOCAB, T = 16384, 256, 32000, 128
DECAY = 0.97
N_CORES = 8
NS = N // N_CORES           # 2048 rows per core
NT = NS // 128              # 16 tiles of 128
NQ = NS // 512              # 4 chunks of 512

N_DUMMY = 6                 # PE warm-up matmuls at t=0

_cache = {}
STAGE = 99


def _build():
    stage = STAGE
    import concourse.bacc as bacc
    import concourse.mybir as mybir
    import concourse.tile as tile

    f32 = mybir.dt.float32
    bf16 = mybir.dt.bfloat16
    AF = mybir.ActivationFunctionType
    ALU = mybir.AluOpType

    nc = bacc.Bacc("TRN2", target_bir_lowering=False, debug=False,
                   num_devices=N_CORES)

    # ---- dram inputs (per-core, host-prepped layouts) ----
    # dxq: 4 quarters, each [128, 1024] = both d-halves of 512 n-rows
    i_dxq = [nc.dram_tensor(f"dxq{q}", [128, 1024], bf16, kind="ExternalInput")
             for q in range(4)]
    i_dyts = nc.dram_tensor("dyts", [128, 2 * NS], bf16, kind="ExternalInput")
    i_ets = nc.dram_tensor("ets", [128, NT * 256], bf16, kind="ExternalInput")
    # cbf = [vts(256) | vwp(256) | vwc_ext(257)]
    i_cbf = nc.dram_tensor("cbf", [128, 769], bf16, kind="ExternalInput")
    # cf32: col0 = ones column; row0 cols 2..130 = ones row
    i_cf32 = nc.dram_tensor("cf32", [128, 130], f32, kind="ExternalInput")

    o_x = nc.dram_tensor("out_x", [NS], f32, kind="ExternalOutput")
    o_y = nc.dram_tensor("out_y", [NS], bf16, kind="ExternalOutput")
    o_vs = nc.dram_tensor("out_vs", [256], f32, kind="ExternalOutput")
    o_s = nc.dram_tensor("out_s", [1], f32, kind="ExternalOutput")
    o_rho = nc.dram_tensor("out_rho", [256, NS], bf16, kind="ExternalOutput")

    with tile.TileContext(nc) as tc:
        with (
            tc.tile_pool(name="persist", bufs=1) as pp,
            tc.tile_pool(name="work", bufs=2) as wp,
            tc.tile_pool(name="psrc", bufs=3, space="PSUM") as psrc,
            tc.tile_pool(name="psrt", bufs=2, space="PSUM") as psrt,
            tc.tile_pool(name="psha", bufs=1, space="PSUM") as psha,
            tc.tile_pool(name="psb", bufs=1, space="PSUM") as psb,
            tc.tile_pool(name="psx", bufs=1, space="PSUM") as psx,
            tc.tile_pool(name="dram", bufs=1, space="DRAM") as dram,
        ):
            # ---------------- loads ----------------
            dxts = pp.tile([128, 4096], bf16)       # [q0d0 q0d1 q1d0 q1d1 ..]
            cbf = pp.tile([128, 769], bf16)
            cf32 = pp.tile([128, 130], f32)
            dyts = pp.tile([128, 2 * NS], bf16)
            ets = pp.tile([128, NT * 256], bf16)
            dm = pp.tile([128, 512], bf16)          # dummy-warmup operand

            # SP: cbf first (gates all matmuls), then dx quarters.
            # Pool: dx quarters + ets.  Act: kept clear for the early relus
            # (the framework's act-table load head-blocks it anyway).
            nc.sync.dma_start(cbf[:], i_cbf[:])
            nc.gpsimd.dma_start(dxts[:, 1024:2048], i_dxq[1][:])
            nc.sync.dma_start(dxts[:, 0:1024], i_dxq[0][:])
            nc.gpsimd.dma_start(dxts[:, 2048:3072], i_dxq[2][:])
            nc.sync.dma_start(dxts[:, 3072:4096], i_dxq[3][:])
            nc.vector.memset(dm[:], 0.0)
            # bulk loads for the post-AR phases; queues are free meanwhile
            nc.sync.dma_start(dyts[:], i_dyts[:])
            nc.gpsimd.dma_start(ets[:], i_ets[:])

            vts = cbf[:, 0:256]
            vwp = cbf[:, 256:512]
            vwc = cbf[:, 512:769]      # [128, 257] incl. mean column
            ones_col = cf32[:, 0:1]
            ones_row = cf32[0:1, 2:130]

            # ---------------- warm-ups ----------------
            # PE p-state ramp; plus a dummy Sqrt so the single act-table
            # load at t~1.5us picks a sqrt-capable set (they all contain
            # Relu/Copy/Square too) instead of a mid-kernel table swap.
            dmsq = pp.tile([1, 1], f32)
            nc.scalar.activation(dmsq[:], dm[0:1, 0:1], AF.Sqrt)
            for k in range(N_DUMMY):
                dm_ps = psrt.tile([128, 256], f32, tag="rt")
                nc.tensor.matmul(dm_ps[:], lhsT=dm[:, 0:128],
                                 rhs=dm[:, 0:256], start=True, stop=True)

            # ---------------- phase A: rcols -> h -> a* ----------------
            # rcols_i = relu(Dx_i V^T) [128n, 128T], fp32 in SBUF;
            # relu accum gives xfcol columns for free.  Per-engine xf
            # accumulator tiles avoid cross-engine WAW serialization.
            # DVE relu is ~2x cheaper per tile (258 vs 479 ns); give Act a
            # few early-group tiles only.
            ACT_TILES = {6, 7, 2, 3, 11}
            rcols = pp.tile([128, NT * 128], f32)
            xfA = pp.tile([128, NT], f32)
            xfB = pp.tile([128, NT], f32)
            xf_of = lambda i: (xfB if i in ACT_TILES else xfA)[:, i:i + 1]
            qorder = [1, 0, 2, 3]
            for gi, q in enumerate(qorder):
                rc_ps = psrc.tile([128, 512], f32, tag="rc")
                for j in range(4):
                    i = q * 4 + j
                    for c in range(2):
                        nc.tensor.matmul(
                            rc_ps[:, j * 128:(j + 1) * 128],
                            lhsT=dxts[:, q * 1024 + c * 512 + j * 128:
                                      q * 1024 + c * 512 + (j + 1) * 128],
                            rhs=vts[:, c * 128:(c + 1) * 128],
                            start=(c == 0), stop=(c == 1))
                for j in range(4):
                    i = q * 4 + j
                    src = rc_ps[:, j * 128:(j + 1) * 128]
                    dst = rcols[:, i * 128:(i + 1) * 128]
                    if i in ACT_TILES:
                        nc.scalar.activation(dst, src, AF.Relu,
                                             accum_out=xf_of(i))
                    else:
                        nc.vector.tensor_scalar(
                            dst, src, 0.0, None, ALU.max, op1=ALU.add,
                            accum_out=xf_of(i))

            if stage >= 2:
                # h = sum_i rcols_i^T xfcol_i  (one PSUM bank accum chain)
                h_ps = psha.tile([128, 1], f32, tag="ha")
                for gi, q in enumerate(qorder):
                    for j in range(4):
                        i = q * 4 + j
                        nc.tensor.matmul(
                            h_ps[:],
                            lhsT=rcols[:, i * 128:(i + 1) * 128],
                            rhs=xf_of(i),
                            start=(gi == 0 and j == 0),
                            stop=(gi == 3 and j == 3))
                h_sb = pp.tile([128, 1], bf16)
                nc.vector.tensor_copy(h_sb[:], h_ps[:])
                # zero column pinned behind h: rt relus take it as operand so
                # the scheduler cannot slot them into the h-critical chain
                zA = pp.tile([128, 1], f32)
                nc.vector.tensor_scalar_mul(zA[:], h_sb[:], 0.0)
                # a_ext = vwc_ext^T h : [1, 257] (col 256 = mean of a)
                a_ps = psha.tile([1, 257], f32, tag="ha")
                nc.tensor.matmul(a_ps[:], lhsT=h_sb[:], rhs=vwc[:],
                                 start=True, stop=True)
                a_sb = pp.tile([1, 257], f32)
                nc.scalar.activation(a_sb[:], a_ps[:], AF.Copy)
                # combined x_f (for o_x and the y elementwise product)
                xfcol = xfA
                nc.gpsimd.tensor_copy(xfcol[:, 2:4], xfB[:, 2:4])
                nc.gpsimd.tensor_copy(xfcol[:, 6:8], xfB[:, 6:8])
                nc.gpsimd.tensor_copy(xfcol[:, 11:12], xfB[:, 11:12])
                nc.scalar.dma_start(o_x[:].rearrange("(i p) -> p i", p=128),
                                    xfcol[:])
                nc.scalar.dma_start(cf32[:], i_cf32[:])

            if stage >= 3:
                # ---- AllReduce #1 (a_ext) ----
                a_in = dram.tile([1, 257], f32)
                a_out = dram.tile([1, 257], f32)
                nc.sync.dma_start(a_in[:], a_sb[:])
                nc.gpsimd.collective_compute(
                    "AllReduce", ALU.add,
                    replica_groups=[list(range(N_CORES))],
                    ins=[a_in.opt()], outs=[a_out.opt()],
                )

            # rt = relu(V Dx^T) [T, n] bf16 (only needed for rho in phase B;
            # relus read zA so they run after the h chain, pre-AR slack)
            rt = pp.tile([128, NS], bf16)
            for q in range(NQ):
                rt_ps = psrt.tile([128, 512], f32, tag="rt")
                for c in range(2):
                    nc.tensor.matmul(
                        rt_ps[:],
                        lhsT=vts[:, c * 128:(c + 1) * 128],
                        rhs=dxts[:, q * 1024 + c * 512: q * 1024 + (c + 1) * 512],
                        start=(c == 0), stop=(c == 1))
                if q % 2 == 0:
                    nc.vector.tensor_scalar(rt[:, q * 512:(q + 1) * 512],
                                            rt_ps[:], zA[:], None, ALU.max)
                else:
                    nc.scalar.activation(rt[:, q * 512:(q + 1) * 512],
                                         rt_ps[:], AF.Relu, bias=zA[:])

            if stage >= 4:
                # ---- phase B: everything behind AllReduce #1 ----
                # Since relu(z/s) = relu(z)/s and the final layernorm is
                # scale-invariant, the 1/std factor of ln(a*) cancels in
                # out_vs; y is computed unscaled and rescaled on the host
                # via the exported out_s.  Dy@ln(a) then reduces to
                # Dy@(a - m), with m folded into the AllReduce payload.
                acol = pp.tile([128, 2], f32)
                mrow = pp.tile([1, 1], f32)
                nc.sync.dma_start(
                    acol[:], a_out[0:1, 0:256].rearrange("a (c p) -> (a p) c", p=128))
                nc.scalar.dma_start(mrow[:], a_out[0:1, 256:257])
                # broadcast m across partitions via a rank-1 matmul
                mb_ps = psx.tile([128, 1], f32, tag="x")
                nc.tensor.matmul(mb_ps[:], lhsT=ones_row[:], rhs=mrow[:],
                                 start=True, stop=True)
                m_col = pp.tile([128, 1], f32)
                nc.vector.tensor_copy(m_col[:], mb_ps[:])
                cenb = pp.tile([128, 2], bf16)
                nc.vector.tensor_scalar(cenb[:], acol[:], m_col[:], None,
                                        ALU.subtract)
                # rho inputs re-materialized with a data dependency on the
                # AllReduce result, so the scheduler cannot hoist the rho
                # chain into the AllReduce window (whose tail must stay
                # quiet for the collective to run at full HW speed).
                zcol = pp.tile([128, 1], f32)
                nc.gpsimd.tensor_scalar_mul(zcol[:], acol[:, 0:1], 0.0)
                vwp_w = pp.tile([128, 256], bf16)
                nc.gpsimd.tensor_scalar(vwp_w[:], vwp, zcol[:], None, ALU.add)

                # yc~ = Dy @ (a - m)  [128, 16]
                yca_ps = psb.tile([128, NT], f32, tag="b")
                for i in range(NT):
                    for c in range(2):
                        nc.tensor.matmul(
                            yca_ps[:, i:i + 1],
                            lhsT=dyts[:, c * NS + i * 128: c * NS + (i + 1) * 128],
                            rhs=cenb[:, c:c + 1],
                            start=(c == 0), stop=(c == 1))
                ycr = pp.tile([128, NT], f32)
                nc.vector.tensor_scalar_max(ycr[:], yca_ps[:], 0.0)
                y = pp.tile([128, NT], bf16)
                nc.vector.tensor_mul(y[:], ycr[:], xfcol[:])
                nc.scalar.dma_start(o_y[:].rearrange("(i p) -> p i", p=128),
                                    y[:])

                # sigma side-chain (only for the host rescale of y):
                # std = sqrt((sum a^2 - 256 m^2)/255), exported as out_s
                sq = pp.tile([128, 2], f32)
                pssq = pp.tile([128, 1], f32)
                nc.vector.tensor_tensor_reduce(
                    out=sq[:], in0=acol[:], in1=acol[:], scale=1.0, scalar=0.0,
                    op0=ALU.mult, op1=ALU.add, accum_out=pssq[:])
                ssq_ps = psx.tile([1, 1], f32, tag="x")
                nc.tensor.matmul(ssq_ps[:], lhsT=pssq[:], rhs=ones_col[:],
                                 start=True, stop=True)
                msq = pp.tile([1, 1], f32)
                msq_o = pp.tile([1, 1], f32)
                nc.vector.tensor_tensor_reduce(
                    out=msq_o[:], in0=mrow[:], in1=mrow[:],
                    scale=-256.0 / 255.0, scalar=0.0,
                    op0=ALU.mult, op1=ALU.add, accum_out=msq[:])
                std = pp.tile([1, 1], f32)
                nc.scalar.activation(std[:], ssq_ps[:], AF.Sqrt,
                                     bias=msq[:], scale=1.0 / 255.0)
                nc.scalar.dma_start(o_s[:].rearrange("(a b) -> a b", a=1),
                                    std[:])

            if stage >= 5:
                # rho = vwp^T @ rt  [256, n]; pinned to phase B via vwp_w.
                # Fine-grained copy->store pipeline: each [128,512] slice is
                # stored as soon as its PSUM->SBUF copy lands, all stores on
                # the SP queue in completion order so they retire before
                # AllReduce #2 issues.
                rho_sb0 = pp.tile([128, NS], bf16)
                rho_sb1 = pp.tile([128, NS], bf16)
                rho_sb = [rho_sb0, rho_sb1]
                for q in range(NQ):
                    for dc in range(2):
                        rho_ps = psrc.tile([128, 512], f32, tag="rc")
                        nc.tensor.matmul(rho_ps[:],
                                         lhsT=vwp_w[:, dc * 128:(dc + 1) * 128],
                                         rhs=rt[:, q * 512:(q + 1) * 512],
                                         start=True, stop=True)
                        dst = rho_sb[dc][:, q * 512:(q + 1) * 512]
                        if dc == 0:
                            nc.scalar.activation(dst, rho_ps[:], AF.Copy)
                        else:
                            nc.vector.tensor_copy(dst, rho_ps[:])
                        nc.sync.dma_start(
                            o_rho[dc * 128:(dc + 1) * 128,
                                  q * 512:(q + 1) * 512], dst)

            if stage >= 6:
                # vs_partial = y^T E^T : [1, 256] + pre-reduced mean col
                vs_ps = psb.tile([1, 256], f32, tag="b")
                for i in range(NT):
                    nc.tensor.matmul(vs_ps[:],
                                     lhsT=y[:, i:i + 1],
                                     rhs=ets[:, i * 256:(i + 1) * 256],
                                     start=(i == 0), stop=(i == NT - 1))
                # separate tiles per engine (same-tile writes from two
                # engines serialize via WAW otherwise)
                vs_sb0 = pp.tile([1, 256], f32)
                vs_m = pp.tile([1, 1], f32)
                vs_scr = pp.tile([1, 256], f32)
                nc.vector.tensor_copy(vs_sb0[:], vs_ps[:])
                nc.scalar.activation(vs_scr[:], vs_ps[:], AF.Copy,
                                     scale=1.0 / 256.0, accum_out=vs_m[:])

                # ---- AllReduce #2 (vs_ext) ----
                vs_in = dram.tile([1, 257], f32)
                vs_out = dram.tile([1, 257], f32)
                nc.sync.dma_start(vs_in[0:1, 0:256], vs_sb0[:])
                nc.scalar.dma_start(vs_in[0:1, 256:257], vs_m[:])
                nc.gpsimd.collective_compute(
                    "AllReduce", ALU.add,
                    replica_groups=[list(range(N_CORES))],
                    ins=[vs_in.opt()], outs=[vs_out.opt()],
                )

            if stage >= 7:
                # ---- phase C: final layernorm of vs ----
                vrow = pp.tile([1, 257], f32)
                nc.sync.dma_start(vrow[:], vs_out[:])
                cen = pp.tile([1, 256], f32)
                nc.vector.tensor_scalar(cen[:], vrow[:, 0:256],
                                        vrow[:, 256:257], None, ALU.subtract)
                vsq = pp.tile([1, 256], f32)
                vssq = pp.tile([1, 1], f32)
                nc.vector.tensor_tensor_reduce(
                    out=vsq[:], in0=cen[:], in1=cen[:], scale=1.0 / 255.0,
                    scalar=0.0, op0=ALU.mult, op1=ALU.add, accum_out=vssq[:])
                vstd = pp.tile([1, 1], f32)
                nc.scalar.activation(vstd[:], vssq[:], AF.Sqrt)
                vinv = pp.tile([1, 1], f32)
                nc.vector.reciprocal(vinv[:], vstd[:])
                vsln = pp.tile([1, 256], f32)
                nc.vector.tensor_scalar(vsln[:], cen[:], vinv[:], None,
                                        ALU.mult)
                nc.sync.dma_start(o_vs[:].rearrange("(a b) -> a b", a=1),
                                  vsln[:])

    nc.finalize()
    return nc


def _host_prep(E, Dx, Dy, token_emb, tokens):
    import ml_dtypes
    bf = ml_dtypes.bfloat16

    E = np.asarray(E, dtype=np.float32)
    Dx = np.asarray(Dx, dtype=np.float32)
    Dy = np.asarray(Dy, dtype=np.float32)
    token_emb = np.asarray(token_emb, dtype=np.float32)
    tokens = np.asarray(tokens).astype(np.int64)

    v = np.ascontiguousarray(token_emb[tokens])          # [T, d]
    j = np.arange(T)
    w = (DECAY ** ((T - 1) - j)).astype(np.float32)
    w[T - 1] = 0.0
    wp = (DECAY ** (T - j)).astype(np.float32)
    u = np.triu(np.ones((T, T), dtype=np.float32))
    vwc = u @ (v * w[:, None])                           # [T, d]
    vwc_ext = np.concatenate([vwc, vwc.sum(1, keepdims=True) / 256.0], axis=1)
    vwp = u @ (v * wp[:, None])                          # [T, d]
    vts = np.concatenate([v[:, :128].T, v[:, 128:].T], axis=1)   # [128, 256]

    # vwp/vwc are [T, d] with T=128 partitions already
    cbf = np.ascontiguousarray(np.concatenate([vts, vwp, vwc_ext], axis=1))

    cf32 = np.zeros((128, 130), dtype=np.float32)
    cf32[:, 0] = 1.0
    cf32[0, 2:130] = 1.0

    in_maps = []
    for k in range(N_CORES):
        sl = slice(k * NS, (k + 1) * NS)
        dx_s = Dx[sl]                                    # [NS, 256]
        dy_s = Dy[sl]
        e_s = E[:, sl]                                   # [256, NS]
        dxq = []
        for q in range(4):
            rs = slice(q * 512, (q + 1) * 512)
            dxq.append(np.ascontiguousarray(np.concatenate(
                [dx_s[rs, :128].T, dx_s[rs, 128:].T], axis=1)).astype(bf))
        dyts = np.concatenate([dy_s[:, :128].T, dy_s[:, 128:].T], axis=1)
        ets = np.concatenate(
            [e_s[:, i * 128:(i + 1) * 128].T for i in range(NT)], axis=1)
        in_maps.append({
            "dxq0": dxq[0], "dxq1": dxq[1], "dxq2": dxq[2], "dxq3": dxq[3],
            "dyts": np.ascontiguousarray(dyts).astype(bf),
            "ets": np.ascontiguousarray(ets).astype(bf),
            "cbf": np.ascontiguousarray(cbf.astype(bf)),
            "cf32": cf32,
        })
    return in_maps


def kernel(E, Dx, Dy, token_emb, tokens, _trace=False):
    from concourse.bass_utils import run_bass_kernel_spmd

    key = ("nc", STAGE)
    if key not in _cache:
        _cache[key] = _build()
    nc = _cache[key]

    in_maps = _host_prep(E, Dx, Dy, token_emb, tokens)
    res = run_bass_kernel_spmd(nc, in_maps, core_ids=list(range(N_CORES)),
                               trace=_trace)
    _cache["last_result"] = res

    r = res.results
    x_full = np.concatenate([r[k]["out_x"] for k in range(N_CORES)])
    # out_y is the unscaled relu(Dy(a-m))*x_f; divide by the exported std
    y_full = np.concatenate(
        [r[k]["out_y"].astype(np.float32) / r[k]["out_s"][0].astype(np.float32)
         for k in range(N_CORES)])
    vs = r[0]["out_vs"]
    rho = np.concatenate(
        [r[k]["out_rho"].astype(np.float32) for k in range(N_CORES)], axis=1)
    return np.concatenate([x_full, y_full, vs, rho.ravel()]).astype(np.float32)
